# revision 13
# baseline (speedup 1.0000x reference)
"""Bidirectional cross-attention kernel for 8 Trainium2 NeuronCores.

Sharding: core c = 2*b + g handles batch b with head-group g (8 of 16 heads).
Each core projects Q/K/V/CV for its 8 heads, computes both softmax
orientations of the shared similarity matrix, and forms the per-head
attention outputs U = attn @ cv and W = context_attn^T @ v (stored
transposed, pre-scaled by the softmax normalizers).  The two cores of a
batch exchange their U/W halves with pairwise AllGathers, after which each
core computes a disjoint 512-column slice of both final projections.

Schedule notes (v2):
 - Phase B (sim+exp+ladder) saturates the scalar engine (128 exps) while
   the PE has spare cycles; the remaining projection matmuls (QT/KT m1-3,
   V, CV) are drip-fed into the PE queue as FILLER between sim/ladder
   steps so both engines run continuously from ~8us onward, instead of a
   serial 74us projection phase with the scalar engine idle.
 - (pair, orientation, head) blocks run sequentially so only one [65,N]
   U/W accumulator is live; PSUM = 2 sim bufs + accumulator + 2 small
   projection psums = exactly 8 banks.
 - Norm path: gpsimd copies the ones-row out of PSUM, DMA roundtrips the
   broadcast, vector does reciprocal + (psum x recip) -> bf16; no
   [65,1024] staging copies.
 - Pair 3's exchanges are split (U, W-head0, W-head1) so only 8 K=64
   matmuls + adds depend on the last gather.
"""

import os
import sys
from collections import deque

import numpy as np

for _p in ("/opt/trn_rl_repo", "/root/.axon_site/_ro/trn_rl_repo"):
    if os.path.isdir(_p) and _p not in sys.path:
        sys.path.append(_p)

import ml_dtypes  # noqa: E402
import concourse.bass as bass  # noqa: E402
import concourse.mybir as mybir  # noqa: E402
import concourse.tile as tile  # noqa: E402
from concourse import bacc  # noqa: E402
from concourse.bass_utils import run_bass_kernel_spmd  # noqa: E402

B, N, DIM = 4, 1024, 1024
H, DH = 16, 64
HL = 8            # heads per core
IL = HL * DH      # local inner width (512)
COLS = 512        # output columns per core
P = 128
PAIRS = HL // 2   # head pairs per core
KCH = DIM // P    # contraction chunks (8)
ICH = N // P      # sequence chunks (8)
SCALE = DH ** -0.5
GROUPS = [[0, 1], [2, 3], [4, 5], [6, 7]]
hs = 65           # head stride in V/CV tiles (64 values + ones column)

F32 = mybir.dt.float32
BF16 = mybir.dt.bfloat16
EXP = mybir.ActivationFunctionType.Exp

_CACHED_NC = None


def _build_nc():
    nc = bacc.Bacc("TRN2", target_bir_lowering=False, debug=False, num_devices=8)

    T = {}
    for nm, shape, dt in (
            ("xT", [DIM, N], BF16), ("ctxT", [DIM, N], BF16),
            ("wqk", [DIM, IL], BF16), ("wv", [DIM, IL], BF16),
            ("cwqk", [DIM, IL], BF16), ("cwv", [DIM, IL], BF16),
            ("wout", [DIM, COLS], BF16), ("cwout", [DIM, COLS], BF16),
            ("bout", [1, COLS], F32), ("cbout", [1, COLS], F32)):
        T[nm] = nc.dram_tensor(nm, shape, dt, kind="ExternalInput")
    T["out_cols"] = nc.dram_tensor("out_cols", [N, COLS], F32, kind="ExternalOutput")
    T["ctx_cols"] = nc.dram_tensor("ctx_cols", [N, COLS], F32, kind="ExternalOutput")

    with tile.TileContext(nc) as tc:
        with tc.tile_pool(name="dram", bufs=1, space="DRAM") as dpool:
            T["uwl"] = [dpool.tile([256, N], BF16, tag=f"uwl{p}", name=f"uwl{p}")
                        for p in range(4)]
            T["uwa"] = [dpool.tile([512, N], BF16, tag=f"uwa{p}", name=f"uwa{p}")
                        for p in range(3)]
            T["uwa3u"] = dpool.tile([256, N], BF16, tag="uwa3u", name="uwa3u")
            T["uwa3wh"] = [dpool.tile([128, N], BF16, tag=f"uwa3wh{h}",
                                      name=f"uwa3wh{h}")
                           for h in range(2)]
            T["normd"] = dpool.tile([16, N], F32, tag="normd", name="normd")
            _build_body(nc, tc, T)
    nc.compile()
    if os.environ.get("KERNEL_LDW_DEDUP", "1") == "1":
        _dedupe_ldweights(nc)
    return nc


def _dedupe_ldweights(nc):
    """Drop PE Ldweights that reload the exact weights already resident."""
    def sig(i):
        a = i.ins[0]
        return (a.memref, a.offset, str(a.ap), str(a.dtype),
                str(i.tile_position), str(i.tile_size),
                str(i.perf_mode), str(i.is_transpose))

    removed = 0
    for fn in nc.m.functions:
        for bb in fn.blocks:
            last = None
            keep = []
            for i in bb.instructions:
                if isinstance(i, mybir.InstLdweights):
                    s = sig(i)
                    si = i.sync_info
                    if s == last and (si is None or
                                      (not si.on_wait and not si.on_update)):
                        removed += 1
                        continue
                    last = s
                elif isinstance(i, mybir.InstMatmult):
                    pass
                elif getattr(i, "engine", None) == mybir.EngineType.PE:
                    last = None
                keep.append(i)
            if removed:
                bb.instructions = keep
    return removed


def _build_body(nc, tc, T):
    from contextlib import ExitStack
    stack = ExitStack()       # pools that live to the end
    bstack = ExitStack()      # pools released before phase C
    pqk = stack.enter_context(tc.tile_pool(name="pqk", bufs=1))
    pv = stack.enter_context(tc.tile_pool(name="pv", bufs=1))
    pf = stack.enter_context(tc.tile_pool(name="pf", bufs=1))
    pu = stack.enter_context(tc.tile_pool(name="pu", bufs=1))
    pe = stack.enter_context(tc.tile_pool(name="pe", bufs=4))
    pn = stack.enter_context(tc.tile_pool(name="pn", bufs=2))
    pn1 = stack.enter_context(tc.tile_pool(name="pn1", bufs=1))
    psB = bstack.enter_context(tc.tile_pool(name="psB", bufs=1, space="PSUM"))
    pw = bstack.enter_context(tc.tile_pool(name="pw", bufs=1))
    pin = bstack.enter_context(tc.tile_pool(name="pin", bufs=1))

    # ---------------- input DMA ----------------
    # critical path: wqk + xT (sync queue) and cwqk + ctxT (scalar queue,
    # which is idle until phase B starts).
    wqk_t, xt, cwqk_t, ct = [], [], [], []
    for k in range(KCH):
        w = pw.tile([P, IL], BF16, tag=f"wqk{k}", name=f"wqk{k}")
        nc.sync.dma_start(w[:], T["wqk"][k * P:(k + 1) * P, :])
        wqk_t.append(w)
        t = pin.tile([P, N], BF16, tag=f"xT{k}", name=f"xt{k}")
        nc.sync.dma_start(t[:], T["xT"][k * P:(k + 1) * P, :])
        xt.append(t)
        w = pw.tile([P, IL], BF16, tag=f"cwqk{k}", name=f"cwqk{k}")
        nc.scalar.dma_start(w[:], T["cwqk"][k * P:(k + 1) * P, :])
        cwqk_t.append(w)
        t = pin.tile([P, N], BF16, tag=f"cT{k}", name=f"ct{k}")
        nc.scalar.dma_start(t[:], T["ctxT"][k * P:(k + 1) * P, :])
        ct.append(t)
    wv_t, cwv_t = [], []
    for k in range(KCH):
        w = pw.tile([P, IL], BF16, tag=f"cwv{k}", name=f"cwv{k}")
        nc.sync.dma_start(w[:], T["cwv"][k * P:(k + 1) * P, :])
        cwv_t.append(w)
    for k in range(KCH):
        w = pw.tile([P, IL], BF16, tag=f"wv{k}", name=f"wv{k}")
        nc.sync.dma_start(w[:], T["wv"][k * P:(k + 1) * P, :])
        wv_t.append(w)
    # output-side weights/biases (needed in phase C only)
    bout_bc = pf.tile([P, COLS], F32, tag="bb")
    nc.scalar.dma_start(bout_bc[:], T["bout"][:].to_broadcast((P, COLS)))
    cbout_bc = pf.tile([P, COLS], F32, tag="cbb")
    nc.scalar.dma_start(cbout_bc[:], T["cbout"][:].to_broadcast((P, COLS)))
    wout_sb, cwout_sb = [], []
    for k in range(KCH):
        t = pf.tile([P, COLS], BF16, tag=f"wo{k}")
        nc.scalar.dma_start(t[:], T["wout"][k * P:(k + 1) * P, :])
        wout_sb.append(t)
        t = pf.tile([P, COLS], BF16, tag=f"cwo{k}")
        nc.scalar.dma_start(t[:], T["cwout"][k * P:(k + 1) * P, :])
        cwout_sb.append(t)

    # ---------------- projection emitters (filler items) ----------------
    QT = [None] * PAIRS   # QT[m] = (pa, pb): head A rows 0:64 / head B 64:128
    KT = [None] * PAIRS
    V = [None] * ICH      # [128, HL*hs] bf16, ones col per head
    CV = [None] * ICH

    def projT_items(src, wtiles, store, m, tag):
        pa = pqk.tile([P, N], BF16, tag=f"{tag}a{m}")
        pb = pqk.tile([P, N], BF16, tag=f"{tag}b{m}")
        store[m] = (pa, pb)
        items = [lambda: (nc.vector.memset(pa[DH:P, :], 0.0),
                          nc.vector.memset(pb[0:DH, :], 0.0))]
        for half in range(2):
            ps = psB.tile([P, COLS], F32, tag="pt", name=f"pt_{tag}{m}_{half}")
            lo = half * COLS

            def mk(k, ps=ps, lo=lo):
                def it():
                    nc.tensor.matmul(ps[:], wtiles[k][:, m * P:(m + 1) * P],
                                     src[k][:, lo:lo + COLS],
                                     start=(k == 0), stop=(k == KCH - 1))
                    if k == KCH - 1:
                        nc.vector.tensor_copy(pa[0:DH, lo:lo + COLS],
                                              ps[0:DH, :])
                        nc.vector.tensor_copy(pb[DH:P, lo:lo + COLS],
                                              ps[DH:P, :])
                return it
            items.extend(mk(k) for k in range(KCH))
        return items

    def projV_items(src, wtiles, store, ic, tag):
        o = pv.tile([P, HL * hs], BF16, tag=f"{tag}{ic}")
        store[ic] = o
        ps = psB.tile([P, IL], F32, tag="pv", name=f"pv_{tag}{ic}")
        items = []

        def mk(k):
            def it():
                nc.tensor.matmul(ps[:], src[k][:, ic * P:(ic + 1) * P],
                                 wtiles[k][:],
                                 start=(k == 0), stop=(k == KCH - 1))
                if k == KCH - 1:
                    dst = o[:].rearrange("p (h e) -> p h e", e=hs)
                    nc.vector.tensor_copy(
                        dst[:, :, 0:DH],
                        ps[:].rearrange("p (h e) -> p h e", e=DH))
                    nc.vector.memset(dst[:, :, DH:hs], 1.0)
            return it
        items.extend(mk(k) for k in range(KCH))
        return items

    # resource name -> remaining items; drip order for background filling
    res = {}
    for ic in range(ICH):
        res[f"cv{ic}"] = projV_items(ct, cwv_t, CV, ic, "cv")
    for ic in range(ICH):
        res[f"v{ic}"] = projV_items(xt, wv_t, V, ic, "v")
    for m in (1, 2, 3):
        res[f"kt{m}"] = projT_items(ct, cwqk_t, KT, m, "kt")
        res[f"qt{m}"] = projT_items(xt, wqk_t, QT, m, "qt")
    drip = deque(
        [f"cv{ic}" for ic in range(ICH)] + ["kt1", "qt1"] +
        [f"v{ic}" for ic in range(ICH)] +
        ["kt2", "qt2", "kt3", "qt3"])

    def require(name):
        for it in res.pop(name, ()):
            it()

    def emit_fillers(n):
        done = 0
        while done < n and drip:
            lst = res.get(drip[0])
            if not lst:
                res.pop(drip[0], None)
                drip.popleft()
                continue
            lst.pop(0)()
            done += 1

    def drain_fillers():
        while drip:
            emit_fillers(len(drip) * 32)

    # ---------------- prelude: QT/KT m0 ----------------
    for it in projT_items(xt, wqk_t, QT, 0, "qt"):
        it()
    for it in projT_items(ct, cwqk_t, KT, 0, "kt"):
        it()

    # ---------------- phase B ----------------
    u_sb = [None] * KCH
    w_sb = [None] * KCH

    def norm(p, orient, hh, acc):
        """acc: [65,N] psum; rows 0:64 data, row 64 normalizer."""
        slot = 4 * p + 2 * orient + hh
        nrow = pn1.tile([1, N], F32, tag="nrow")
        nc.vector.tensor_copy(nrow[:], acc[DH:DH + 1, :])
        nc.sync.dma_start(T["normd"][slot:slot + 1, :], nrow[:])
        rbc = pn.tile([DH, N], F32, tag="rbc")
        nc.sync.dma_start(
            rbc[:], T["normd"][slot:slot + 1, :].to_broadcast((DH, N)))
        nc.vector.reciprocal_approx_fast(rbc[:], rbc[:])
        ubf = pn.tile([DH, N], BF16, tag="ubf")
        nc.vector.tensor_mul(ubf[:], acc[0:DH, :], rbc[:])
        r0 = orient * 128 + hh * DH
        nc.sync.dma_start(T["uwl"][p][r0:r0 + DH, :], ubf[:])

    def block(p, orient, hh):
        """One (pair, orientation, head) unit: 8 sim+exp steps with the
        U/W ladder at lag 2 and fillers padding the PE."""
        h = 2 * p + hh
        require(f"qt{p}")
        require(f"kt{p}")
        if orient == 0:   # U: simT (j on partitions), ladder vs CV
            lhsT_full, rhs_full = KT[p][hh], QT[p][hh]
            lad = CV
        else:             # W: sim (i on partitions), ladder vs V
            lhsT_full, rhs_full = QT[p][hh], KT[p][hh]
            lad = V
        acc = psB.tile([P, N], F32, tag="uw", name=f"uw{p}_{orient}_{hh}")
        E = [None] * ICH

        def ladder(jc):
            require(f"{'cv' if orient == 0 else 'v'}{jc}")
            lt = lad[jc][:, h * hs:(h + 1) * hs]
            for half in range(2):
                nc.tensor.matmul(
                    acc[0:hs, half * COLS:(half + 1) * COLS],
                    lt, E[jc][:, half * COLS:(half + 1) * COLS],
                    start=(jc == 0), stop=(jc == ICH - 1))

        for jc in range(ICH):
            ps = psB.tile([P, N], F32, tag=f"sim{jc & 1}")
            lhsT = lhsT_full[:, jc * P:(jc + 1) * P]
            for half in range(2):
                nc.tensor.matmul(ps[:, half * COLS:(half + 1) * COLS],
                                 lhsT, rhs_full[:, half * COLS:(half + 1) * COLS],
                                 start=True, stop=True)
            e = pe.tile([P, N], BF16, tag="E")
            nc.scalar.activation(e[:], ps[:], EXP, scale=SCALE)
            E[jc] = e
            if jc >= 2:
                ladder(jc - 2)
            emit_fillers(2)
        ladder(ICH - 2)
        ladder(ICH - 1)
        norm(p, orient, hh, acc)

    def load_pair(p):
        rows = ((0, u_sb, "u", 2 * p), (256, u_sb, "u", 2 * p + 1),
                (128, w_sb, "w", 2 * p), (384, w_sb, "w", 2 * p + 1))
        for row, arr, nm, k in rows:
            t = pu.tile([P, N], BF16, tag=f"{nm}sb{k}", name=f"{nm}sb{k}")
            nc.sync.dma_start(t[:], T["uwa"][p][row:row + P, :])
            arr[k] = t

    for p in range(PAIRS):
        block(p, 0, 0)
        block(p, 0, 1)
        if p == 3:
            nc.gpsimd.collective_compute(
                "AllGather", mybir.AluOpType.bypass,
                replica_groups=GROUPS,
                ins=[T["uwl"][3][0:128, :]],
                outs=[T["uwa3u"][:]],
            )
            for j in range(2):
                t = pu.tile([P, N], BF16, tag=f"usb{6 + j}", name=f"usb{6 + j}")
                nc.sync.dma_start(t[:], T["uwa3u"][j * P:(j + 1) * P, :])
                u_sb[6 + j] = t
        block(p, 1, 0)
        if p == 3:
            nc.gpsimd.collective_compute(
                "AllGather", mybir.AluOpType.bypass,
                replica_groups=GROUPS,
                ins=[T["uwl"][3][128:128 + DH, :]],
                outs=[T["uwa3wh"][0][:]],
            )
        block(p, 1, 1)
        if p < 3:
            nc.gpsimd.collective_compute(
                "AllGather", mybir.AluOpType.bypass,
                replica_groups=GROUPS,
                ins=[T["uwl"][p][:]],
                outs=[T["uwa"][p][:]],
            )
            load_pair(p)
        else:
            nc.gpsimd.collective_compute(
                "AllGather", mybir.AluOpType.bypass,
                replica_groups=GROUPS,
                ins=[T["uwl"][3][128 + DH:256, :]],
                outs=[T["uwa3wh"][1][:]],
            )
            w6 = pu.tile([P, N], BF16, tag="wsb6")
            w7 = pu.tile([P, N], BF16, tag="wsb7")
            for hh in range(2):
                nc.sync.dma_start(w6[hh * DH:(hh + 1) * DH, :],
                                  T["uwa3wh"][hh][0:DH, :])
                nc.sync.dma_start(w7[hh * DH:(hh + 1) * DH, :],
                                  T["uwa3wh"][hh][DH:P, :])
            w_sb[6], w_sb[7] = w6, w7

    drain_fillers()
    bstack.close()   # free pw/pin SBUF and psB PSUM for phase C

    # ---------------- phase C ----------------
    with tc.tile_pool(name="po", bufs=4) as po, \
         tc.tile_pool(name="psD", bufs=4, space="PSUM") as psD:
        ctx_part = []
        for ic in range(ICH):
            ps = psD.tile([P, COLS], F32, tag="od", name=f"outp{ic}")
            for k in range(KCH):
                nc.tensor.matmul(ps[:], u_sb[k][:, ic * P:(ic + 1) * P],
                                 wout_sb[k][:],
                                 start=(k == 0), stop=(k == KCH - 1))
            o = po.tile([P, COLS], F32, tag="ot")
            nc.vector.tensor_add(o[:], ps[:], bout_bc[:])
            (nc.sync if ic % 2 else nc.scalar).dma_start(
                T["out_cols"][ic * P:(ic + 1) * P, :], o[:])
            # ctx partial: k0..5 full + k6/k7 head-0 rows (K=64)
            ps2 = psD.tile([P, COLS], F32, tag="od", name=f"ctxp{ic}")
            for k in range(6):
                nc.tensor.matmul(ps2[:], w_sb[k][:, ic * P:(ic + 1) * P],
                                 cwout_sb[k][:],
                                 start=(k == 0), stop=False)
            for k in (6, 7):
                nc.tensor.matmul(ps2[:], w_sb[k][0:DH, ic * P:(ic + 1) * P],
                                 cwout_sb[k][0:DH, :],
                                 start=False, stop=(k == 7))
            cp = pu.tile([P, COLS], F32, tag=f"cp{ic}")
            nc.vector.tensor_add(cp[:], ps2[:], cbout_bc[:])
            ctx_part.append(cp)
        for ic in range(ICH):
            ps = psD.tile([P, COLS], F32, tag="od", name=f"ctxf{ic}")
            for k in (6, 7):
                nc.tensor.matmul(ps[:], w_sb[k][DH:P, ic * P:(ic + 1) * P],
                                 cwout_sb[k][DH:P, :],
                                 start=(k == 6), stop=(k == 7))
            o = po.tile([P, COLS], F32, tag="ot")
            nc.vector.tensor_add(o[:], ps[:], ctx_part[ic][:])
            (nc.sync if ic % 2 else nc.scalar).dma_start(
                T["ctx_cols"][ic * P:(ic + 1) * P, :], o[:])
    stack.close()


def _get_nc():
    global _CACHED_NC
    if _CACHED_NC is None:
        _CACHED_NC = _build_nc()
    return _CACHED_NC


def _reorder_rows(w):
    """Reorder [INNER, :] rows to the uw_all K-chunk order (p-major, group X)."""
    chunks = []
    for p in range(4):
        for X in range(2):
            chunks.append(w[X * 512 + p * 128:X * 512 + (p + 1) * 128])
    return np.concatenate(chunks, axis=0)


def kernel(x, context, w_qk, w_v, cw_qk, cw_v, w_out, b_out, cw_out, cb_out):
    x = np.asarray(x, dtype=np.float32)
    context = np.asarray(context, dtype=np.float32)
    w_qk = np.asarray(w_qk, dtype=np.float32)
    w_v = np.asarray(w_v, dtype=np.float32)
    cw_qk = np.asarray(cw_qk, dtype=np.float32)
    cw_v = np.asarray(cw_v, dtype=np.float32)
    w_out_r = _reorder_rows(np.asarray(w_out, dtype=np.float32)).astype(ml_dtypes.bfloat16)
    cw_out_r = _reorder_rows(np.asarray(cw_out, dtype=np.float32)).astype(ml_dtypes.bfloat16)
    b_out = np.asarray(b_out, dtype=np.float32)
    cb_out = np.asarray(cb_out, dtype=np.float32)

    in_maps = []
    for c in range(8):
        b, g = c // 2, c % 2
        sl = slice(g * IL, (g + 1) * IL)
        in_maps.append({
            "xT": np.ascontiguousarray(x[b].T).astype(ml_dtypes.bfloat16),
            "ctxT": np.ascontiguousarray(context[b].T).astype(ml_dtypes.bfloat16),
            "wqk": np.ascontiguousarray(w_qk[:, sl]).astype(ml_dtypes.bfloat16),
            "wv": np.ascontiguousarray(w_v[:, sl]).astype(ml_dtypes.bfloat16),
            "cwqk": np.ascontiguousarray(cw_qk[:, sl]).astype(ml_dtypes.bfloat16),
            "cwv": np.ascontiguousarray(cw_v[:, sl]).astype(ml_dtypes.bfloat16),
            "wout": np.ascontiguousarray(w_out_r[:, sl]),
            "cwout": np.ascontiguousarray(cw_out_r[:, sl]),
            "bout": np.ascontiguousarray(b_out[None, sl]),
            "cbout": np.ascontiguousarray(cb_out[None, sl]),
        })

    nc = _get_nc()
    res = run_bass_kernel_spmd(nc, in_maps, list(range(8)))

    out = np.empty((B, N, DIM), dtype=np.float32)
    ctx_out = np.empty((B, N, DIM), dtype=np.float32)
    for b in range(B):
        out[b, :, 0:COLS] = res.results[2 * b]["out_cols"]
        out[b, :, COLS:] = res.results[2 * b + 1]["out_cols"]
        ctx_out[b, :, 0:COLS] = res.results[2 * b]["ctx_cols"]
        ctx_out[b, :, COLS:] = res.results[2 * b + 1]["ctx_cols"]
    return out, ctx_out


# revision 14
# speedup vs baseline: 1.2860x; 1.2860x over previous
"""Bidirectional cross-attention kernel for 8 Trainium2 NeuronCores.

Sharding: core c = 2*b + g handles batch b with head-group g (8 of 16 heads).
Each core projects Q/K/V/CV for its 8 heads, computes both softmax
orientations of the shared similarity matrix, and forms the per-head
attention outputs U = attn @ cv and W = context_attn^T @ v (stored
transposed, pre-scaled by the softmax normalizers).  The two cores of a
batch exchange their U/W halves with pairwise AllGathers, after which each
core computes a disjoint 512-column slice of both final projections.

Schedule notes (v2):
 - Phase B (sim+exp+ladder) saturates the scalar engine (128 exps) while
   the PE has spare cycles; the remaining projection matmuls (QT/KT m1-3,
   V, CV) are drip-fed into the PE queue as FILLER between sim/ladder
   steps so both engines run continuously from ~8us onward, instead of a
   serial 74us projection phase with the scalar engine idle.
 - (pair, orientation, head) blocks run sequentially so only one [65,N]
   U/W accumulator is live; PSUM = 2 sim bufs + accumulator + 2 small
   projection psums = exactly 8 banks.
 - Norm path: gpsimd copies the ones-row out of PSUM, DMA roundtrips the
   broadcast, vector does reciprocal + (psum x recip) -> bf16; no
   [65,1024] staging copies.
 - Pair 3's exchanges are split (U, W-head0, W-head1) so only 8 K=64
   matmuls + adds depend on the last gather.
"""

import os
import sys
from collections import deque

import numpy as np

for _p in ("/opt/trn_rl_repo", "/root/.axon_site/_ro/trn_rl_repo"):
    if os.path.isdir(_p) and _p not in sys.path:
        sys.path.append(_p)

import ml_dtypes  # noqa: E402
import concourse.bass as bass  # noqa: E402
import concourse.mybir as mybir  # noqa: E402
import concourse.tile as tile  # noqa: E402
from concourse import bacc  # noqa: E402
from concourse.bass_utils import run_bass_kernel_spmd  # noqa: E402

B, N, DIM = 4, 1024, 1024
H, DH = 16, 64
HL = 8            # heads per core
IL = HL * DH      # local inner width (512)
COLS = 512        # output columns per core
P = 128
PAIRS = HL // 2   # head pairs per core
KCH = DIM // P    # contraction chunks (8)
ICH = N // P      # sequence chunks (8)
SCALE = DH ** -0.5
GROUPS = [[0, 1], [2, 3], [4, 5], [6, 7]]
hs = 65           # head stride in V/CV tiles (64 values + ones column)

F32 = mybir.dt.float32
BF16 = mybir.dt.bfloat16
EXP = mybir.ActivationFunctionType.Exp

_CACHED_NC = None


def _build_nc():
    nc = bacc.Bacc("TRN2", target_bir_lowering=False, debug=False, num_devices=8)

    T = {}
    for nm, shape, dt in (
            ("xT", [DIM, N], BF16), ("ctxT", [DIM, N], BF16),
            ("wqk", [DIM, IL], BF16), ("wv", [DIM, IL], BF16),
            ("cwqk", [DIM, IL], BF16), ("cwv", [DIM, IL], BF16),
            ("wout", [DIM, COLS], BF16), ("cwout", [DIM, COLS], BF16),
            ("bout", [1, COLS], F32), ("cbout", [1, COLS], F32)):
        T[nm] = nc.dram_tensor(nm, shape, dt, kind="ExternalInput")
    T["out_cols"] = nc.dram_tensor("out_cols", [N, COLS], F32, kind="ExternalOutput")
    T["ctx_cols"] = nc.dram_tensor("ctx_cols", [N, COLS], F32, kind="ExternalOutput")

    with tile.TileContext(nc) as tc:
        with tc.tile_pool(name="dram", bufs=1, space="DRAM") as dpool:
            T["uwl"] = [dpool.tile([256, N], BF16, tag=f"uwl{p}", name=f"uwl{p}")
                        for p in range(4)]
            T["uwa"] = [dpool.tile([512, N], BF16, tag=f"uwa{p}", name=f"uwa{p}")
                        for p in range(3)]
            T["uwa3u"] = dpool.tile([256, N], BF16, tag="uwa3u", name="uwa3u")
            T["uwa3wh"] = [dpool.tile([128, N], BF16, tag=f"uwa3wh{h}",
                                      name=f"uwa3wh{h}")
                           for h in range(2)]
            T["normd"] = dpool.tile([16, N], F32, tag="normd", name="normd")
            _build_body(nc, tc, T)
    nc.compile()
    if os.environ.get("KERNEL_LDW_DEDUP", "1") == "1":
        _dedupe_ldweights(nc)
    return nc


def _dedupe_ldweights(nc):
    """Drop PE Ldweights that reload the exact weights already resident."""
    def sig(i):
        a = i.ins[0]
        return (a.memref, a.offset, str(a.ap), str(a.dtype),
                str(i.tile_position), str(i.tile_size),
                str(i.perf_mode), str(i.is_transpose))

    removed = 0
    for fn in nc.m.functions:
        for bb in fn.blocks:
            last = None
            keep = []
            for i in bb.instructions:
                if isinstance(i, mybir.InstLdweights):
                    s = sig(i)
                    si = i.sync_info
                    if s == last and (si is None or
                                      (not si.on_wait and not si.on_update)):
                        removed += 1
                        continue
                    last = s
                elif isinstance(i, mybir.InstMatmult):
                    pass
                elif getattr(i, "engine", None) == mybir.EngineType.PE:
                    last = None
                keep.append(i)
            if removed:
                bb.instructions = keep
    return removed


def _build_body(nc, tc, T):
    from contextlib import ExitStack
    stack = ExitStack()       # pools that live to the end
    bstack = ExitStack()      # pools released before phase C
    pqk = stack.enter_context(tc.tile_pool(name="pqk", bufs=1))
    pv = stack.enter_context(tc.tile_pool(name="pv", bufs=1))
    pf = stack.enter_context(tc.tile_pool(name="pf", bufs=1))
    pu = stack.enter_context(tc.tile_pool(name="pu", bufs=1))
    pe = stack.enter_context(tc.tile_pool(name="pe", bufs=4))
    pn = stack.enter_context(tc.tile_pool(name="pn", bufs=2))
    pn1 = stack.enter_context(tc.tile_pool(name="pn1", bufs=1))
    psB = bstack.enter_context(tc.tile_pool(name="psB", bufs=1, space="PSUM"))
    pw = bstack.enter_context(tc.tile_pool(name="pw", bufs=1))
    pin = bstack.enter_context(tc.tile_pool(name="pin", bufs=1))

    # ---------------- input DMA ----------------
    # critical path: wqk + xT (sync queue) and cwqk + ctxT (scalar queue,
    # which is idle until phase B starts).
    wqk_t, xt, cwqk_t, ct = [], [], [], []
    for k in range(KCH):
        w = pw.tile([P, IL], BF16, tag=f"wqk{k}", name=f"wqk{k}")
        nc.sync.dma_start(w[:], T["wqk"][k * P:(k + 1) * P, :])
        wqk_t.append(w)
        t = pin.tile([P, N], BF16, tag=f"xT{k}", name=f"xt{k}")
        nc.sync.dma_start(t[:], T["xT"][k * P:(k + 1) * P, :])
        xt.append(t)
        w = pw.tile([P, IL], BF16, tag=f"cwqk{k}", name=f"cwqk{k}")
        nc.scalar.dma_start(w[:], T["cwqk"][k * P:(k + 1) * P, :])
        cwqk_t.append(w)
        t = pin.tile([P, N], BF16, tag=f"cT{k}", name=f"ct{k}")
        nc.scalar.dma_start(t[:], T["ctxT"][k * P:(k + 1) * P, :])
        ct.append(t)
    wv_t, cwv_t = [], []
    for k in range(KCH):
        w = pw.tile([P, IL], BF16, tag=f"cwv{k}", name=f"cwv{k}")
        nc.gpsimd.dma_start(w[:], T["cwv"][k * P:(k + 1) * P, :])
        cwv_t.append(w)
    for k in range(KCH):
        w = pw.tile([P, IL], BF16, tag=f"wv{k}", name=f"wv{k}")
        nc.gpsimd.dma_start(w[:], T["wv"][k * P:(k + 1) * P, :])
        wv_t.append(w)
    # output-side weights/biases (needed in phase C only)
    bout_bc = pf.tile([P, COLS], F32, tag="bb")
    nc.gpsimd.dma_start(bout_bc[:], T["bout"][:].to_broadcast((P, COLS)))
    cbout_bc = pf.tile([P, COLS], F32, tag="cbb")
    nc.gpsimd.dma_start(cbout_bc[:], T["cbout"][:].to_broadcast((P, COLS)))
    wout_sb, cwout_sb = [], []
    for k in range(KCH):
        t = pf.tile([P, COLS], BF16, tag=f"wo{k}")
        nc.gpsimd.dma_start(t[:], T["wout"][k * P:(k + 1) * P, :])
        wout_sb.append(t)
        t = pf.tile([P, COLS], BF16, tag=f"cwo{k}")
        nc.gpsimd.dma_start(t[:], T["cwout"][k * P:(k + 1) * P, :])
        cwout_sb.append(t)

    # ---------------- projection emitters (filler items) ----------------
    QT = [None] * PAIRS   # QT[m] = (pa, pb): head A rows 0:64 / head B 64:128
    KT = [None] * PAIRS
    V = [None] * ICH      # [128, HL*hs] bf16, ones col per head
    CV = [None] * ICH

    def projT_items(src, wtiles, store, m, tag):
        pa = pqk.tile([P, N], BF16, tag=f"{tag}a{m}")
        pb = pqk.tile([P, N], BF16, tag=f"{tag}b{m}")
        store[m] = (pa, pb)
        items = [lambda: (nc.vector.memset(pa[DH:P, :], 0.0),
                          nc.vector.memset(pb[0:DH, :], 0.0))]
        for half in range(2):
            ps = psB.tile([P, COLS], F32, tag="pt", name=f"pt_{tag}{m}_{half}")
            lo = half * COLS

            def mk(k, ps=ps, lo=lo):
                def it():
                    nc.tensor.matmul(ps[:], wtiles[k][:, m * P:(m + 1) * P],
                                     src[k][:, lo:lo + COLS],
                                     start=(k == 0), stop=(k == KCH - 1))
                    if k == KCH - 1:
                        nc.vector.tensor_copy(pa[0:DH, lo:lo + COLS],
                                              ps[0:DH, :])
                        nc.vector.tensor_copy(pb[DH:P, lo:lo + COLS],
                                              ps[DH:P, :])
                return it
            items.extend(mk(k) for k in range(KCH))
        return items

    def projV_items(src, wtiles, store, ic, tag):
        o = pv.tile([P, HL * hs], BF16, tag=f"{tag}{ic}")
        store[ic] = o
        ps = psB.tile([P, IL], F32, tag="pv", name=f"pv_{tag}{ic}")
        items = []

        def mk(k):
            def it():
                nc.tensor.matmul(ps[:], src[k][:, ic * P:(ic + 1) * P],
                                 wtiles[k][:],
                                 start=(k == 0), stop=(k == KCH - 1))
                if k == KCH - 1:
                    dst = o[:].rearrange("p (h e) -> p h e", e=hs)
                    nc.vector.tensor_copy(
                        dst[:, :, 0:DH],
                        ps[:].rearrange("p (h e) -> p h e", e=DH))
                    nc.vector.memset(dst[:, :, DH:hs], 1.0)
            return it
        items.extend(mk(k) for k in range(KCH))
        return items

    # resource name -> remaining items; drip order for background filling
    res = {}
    for ic in range(ICH):
        res[f"cv{ic}"] = projV_items(ct, cwv_t, CV, ic, "cv")
    for ic in range(ICH):
        res[f"v{ic}"] = projV_items(xt, wv_t, V, ic, "v")
    for m in (1, 2, 3):
        res[f"kt{m}"] = projT_items(ct, cwqk_t, KT, m, "kt")
        res[f"qt{m}"] = projT_items(xt, wqk_t, QT, m, "qt")
    drip = deque(
        [f"cv{ic}" for ic in range(ICH)] + ["kt1", "qt1"] +
        [f"v{ic}" for ic in range(ICH)] +
        ["kt2", "qt2", "kt3", "qt3"])

    def require(name):
        for it in res.pop(name, ()):
            it()

    def emit_fillers(n):
        done = 0
        while done < n and drip:
            lst = res.get(drip[0])
            if not lst:
                res.pop(drip[0], None)
                drip.popleft()
                continue
            lst.pop(0)()
            done += 1

    def drain_fillers():
        while drip:
            emit_fillers(len(drip) * 32)

    # ---------------- prelude: QT/KT m0 ----------------
    for it in projT_items(xt, wqk_t, QT, 0, "qt"):
        it()
    for it in projT_items(ct, cwqk_t, KT, 0, "kt"):
        it()

    # ---------------- phase B ----------------
    u_sb = [None] * KCH
    w_sb = [None] * KCH

    def norm(p, orient, hh, acc):
        """acc: [65,N] psum; rows 0:64 data, row 64 normalizer."""
        slot = 4 * p + 2 * orient + hh
        rst = pn1.tile([DH + 1, N], F32, tag="rst")
        nc.vector.tensor_copy(rst[:], acc[0:DH + 1, :])
        nc.sync.dma_start(T["normd"][slot:slot + 1, :], rst[DH:DH + 1, :])
        rbc = pn.tile([DH, N], F32, tag="rbc")
        nc.sync.dma_start(
            rbc[:], T["normd"][slot:slot + 1, :].to_broadcast((DH, N)))
        nc.vector.reciprocal_approx_fast(rbc[:], rbc[:])
        ubf = pn.tile([DH, N], BF16, tag="ubf")
        nc.vector.tensor_mul(ubf[:], rst[0:DH, :], rbc[:])
        r0 = orient * 128 + hh * DH
        nc.sync.dma_start(T["uwl"][p][r0:r0 + DH, :], ubf[:])

    def block(p, orient, hh):
        """One (pair, orientation, head) unit: 8 sim+exp steps with the
        U/W ladder at lag 2 and fillers padding the PE."""
        h = 2 * p + hh
        require(f"qt{p}")
        require(f"kt{p}")
        if orient == 0:   # U: simT (j on partitions), ladder vs CV
            lhsT_full, rhs_full = KT[p][hh], QT[p][hh]
            lad = CV
        else:             # W: sim (i on partitions), ladder vs V
            lhsT_full, rhs_full = QT[p][hh], KT[p][hh]
            lad = V
        acc = psB.tile([P, N], F32, tag="uw", name=f"uw{p}_{orient}_{hh}")
        E = [None] * ICH

        def ladder(jc):
            require(f"{'cv' if orient == 0 else 'v'}{jc}")
            lt = lad[jc][:, h * hs:(h + 1) * hs]
            for half in range(2):
                nc.tensor.matmul(
                    acc[0:hs, half * COLS:(half + 1) * COLS],
                    lt, E[jc][:, half * COLS:(half + 1) * COLS],
                    start=(jc == 0), stop=(jc == ICH - 1))

        for jc in range(ICH):
            ps = psB.tile([P, N], F32, tag=f"sim{jc & 1}")
            lhsT = lhsT_full[:, jc * P:(jc + 1) * P]
            for half in range(2):
                nc.tensor.matmul(ps[:, half * COLS:(half + 1) * COLS],
                                 lhsT, rhs_full[:, half * COLS:(half + 1) * COLS],
                                 start=True, stop=True)
            e = pe.tile([P, N], BF16, tag="E")
            nc.scalar.activation(e[:], ps[:], EXP, scale=SCALE)
            E[jc] = e
            if jc >= 2:
                ladder(jc - 2)
            emit_fillers(2)
        ladder(ICH - 2)
        ladder(ICH - 1)
        norm(p, orient, hh, acc)

    def load_pair(p):
        rows = ((0, u_sb, "u", 2 * p), (256, u_sb, "u", 2 * p + 1),
                (128, w_sb, "w", 2 * p), (384, w_sb, "w", 2 * p + 1))
        for row, arr, nm, k in rows:
            t = pu.tile([P, N], BF16, tag=f"{nm}sb{k}", name=f"{nm}sb{k}")
            nc.sync.dma_start(t[:], T["uwa"][p][row:row + P, :])
            arr[k] = t

    for p in range(PAIRS):
        block(p, 0, 0)
        block(p, 0, 1)
        if p == 3:
            nc.gpsimd.collective_compute(
                "AllGather", mybir.AluOpType.bypass,
                replica_groups=GROUPS,
                ins=[T["uwl"][3][0:128, :]],
                outs=[T["uwa3u"][:]],
            )
            for j in range(2):
                t = pu.tile([P, N], BF16, tag=f"usb{6 + j}", name=f"usb{6 + j}")
                nc.sync.dma_start(t[:], T["uwa3u"][j * P:(j + 1) * P, :])
                u_sb[6 + j] = t
        block(p, 1, 0)
        if p == 3:
            nc.gpsimd.collective_compute(
                "AllGather", mybir.AluOpType.bypass,
                replica_groups=GROUPS,
                ins=[T["uwl"][3][128:128 + DH, :]],
                outs=[T["uwa3wh"][0][:]],
            )
        block(p, 1, 1)
        if p < 3:
            nc.gpsimd.collective_compute(
                "AllGather", mybir.AluOpType.bypass,
                replica_groups=GROUPS,
                ins=[T["uwl"][p][:]],
                outs=[T["uwa"][p][:]],
            )
            load_pair(p)
        else:
            nc.gpsimd.collective_compute(
                "AllGather", mybir.AluOpType.bypass,
                replica_groups=GROUPS,
                ins=[T["uwl"][3][128 + DH:256, :]],
                outs=[T["uwa3wh"][1][:]],
            )
            w6 = pu.tile([P, N], BF16, tag="wsb6")
            w7 = pu.tile([P, N], BF16, tag="wsb7")
            for hh in range(2):
                nc.sync.dma_start(w6[hh * DH:(hh + 1) * DH, :],
                                  T["uwa3wh"][hh][0:DH, :])
                nc.sync.dma_start(w7[hh * DH:(hh + 1) * DH, :],
                                  T["uwa3wh"][hh][DH:P, :])
            w_sb[6], w_sb[7] = w6, w7

    drain_fillers()
    bstack.close()   # free pw/pin SBUF and psB PSUM for phase C

    # ---------------- phase C ----------------
    with tc.tile_pool(name="po", bufs=4) as po, \
         tc.tile_pool(name="psD", bufs=4, space="PSUM") as psD:
        ctx_part = []
        for ic in range(ICH):
            ps = psD.tile([P, COLS], F32, tag="od", name=f"outp{ic}")
            for k in range(KCH):
                nc.tensor.matmul(ps[:], u_sb[k][:, ic * P:(ic + 1) * P],
                                 wout_sb[k][:],
                                 start=(k == 0), stop=(k == KCH - 1))
            o = po.tile([P, COLS], F32, tag="ot")
            nc.vector.tensor_add(o[:], ps[:], bout_bc[:])
            (nc.sync if ic % 2 else nc.scalar).dma_start(
                T["out_cols"][ic * P:(ic + 1) * P, :], o[:])
            # ctx partial: k0..5 full + k6/k7 head-0 rows (K=64)
            ps2 = psD.tile([P, COLS], F32, tag="od", name=f"ctxp{ic}")
            for k in range(6):
                nc.tensor.matmul(ps2[:], w_sb[k][:, ic * P:(ic + 1) * P],
                                 cwout_sb[k][:],
                                 start=(k == 0), stop=False)
            for k in (6, 7):
                nc.tensor.matmul(ps2[:], w_sb[k][0:DH, ic * P:(ic + 1) * P],
                                 cwout_sb[k][0:DH, :],
                                 start=False, stop=(k == 7))
            cp = pu.tile([P, COLS], F32, tag=f"cp{ic}")
            nc.vector.tensor_add(cp[:], ps2[:], cbout_bc[:])
            ctx_part.append(cp)
        for ic in range(ICH):
            ps = psD.tile([P, COLS], F32, tag="od", name=f"ctxf{ic}")
            for k in (6, 7):
                nc.tensor.matmul(ps[:], w_sb[k][DH:P, ic * P:(ic + 1) * P],
                                 cwout_sb[k][DH:P, :],
                                 start=(k == 6), stop=(k == 7))
            o = po.tile([P, COLS], F32, tag="ot")
            nc.vector.tensor_add(o[:], ps[:], ctx_part[ic][:])
            (nc.sync if ic % 2 else nc.scalar).dma_start(
                T["ctx_cols"][ic * P:(ic + 1) * P, :], o[:])
    stack.close()


def _get_nc():
    global _CACHED_NC
    if _CACHED_NC is None:
        _CACHED_NC = _build_nc()
    return _CACHED_NC


def _reorder_rows(w):
    """Reorder [INNER, :] rows to the uw_all K-chunk order (p-major, group X)."""
    chunks = []
    for p in range(4):
        for X in range(2):
            chunks.append(w[X * 512 + p * 128:X * 512 + (p + 1) * 128])
    return np.concatenate(chunks, axis=0)


def kernel(x, context, w_qk, w_v, cw_qk, cw_v, w_out, b_out, cw_out, cb_out):
    x = np.asarray(x, dtype=np.float32)
    context = np.asarray(context, dtype=np.float32)
    w_qk = np.asarray(w_qk, dtype=np.float32)
    w_v = np.asarray(w_v, dtype=np.float32)
    cw_qk = np.asarray(cw_qk, dtype=np.float32)
    cw_v = np.asarray(cw_v, dtype=np.float32)
    w_out_r = _reorder_rows(np.asarray(w_out, dtype=np.float32)).astype(ml_dtypes.bfloat16)
    cw_out_r = _reorder_rows(np.asarray(cw_out, dtype=np.float32)).astype(ml_dtypes.bfloat16)
    b_out = np.asarray(b_out, dtype=np.float32)
    cb_out = np.asarray(cb_out, dtype=np.float32)

    in_maps = []
    for c in range(8):
        b, g = c // 2, c % 2
        sl = slice(g * IL, (g + 1) * IL)
        in_maps.append({
            "xT": np.ascontiguousarray(x[b].T).astype(ml_dtypes.bfloat16),
            "ctxT": np.ascontiguousarray(context[b].T).astype(ml_dtypes.bfloat16),
            "wqk": np.ascontiguousarray(w_qk[:, sl]).astype(ml_dtypes.bfloat16),
            "wv": np.ascontiguousarray(w_v[:, sl]).astype(ml_dtypes.bfloat16),
            "cwqk": np.ascontiguousarray(cw_qk[:, sl]).astype(ml_dtypes.bfloat16),
            "cwv": np.ascontiguousarray(cw_v[:, sl]).astype(ml_dtypes.bfloat16),
            "wout": np.ascontiguousarray(w_out_r[:, sl]),
            "cwout": np.ascontiguousarray(cw_out_r[:, sl]),
            "bout": np.ascontiguousarray(b_out[None, sl]),
            "cbout": np.ascontiguousarray(cb_out[None, sl]),
        })

    nc = _get_nc()
    res = run_bass_kernel_spmd(nc, in_maps, list(range(8)))

    out = np.empty((B, N, DIM), dtype=np.float32)
    ctx_out = np.empty((B, N, DIM), dtype=np.float32)
    for b in range(B):
        out[b, :, 0:COLS] = res.results[2 * b]["out_cols"]
        out[b, :, COLS:] = res.results[2 * b + 1]["out_cols"]
        ctx_out[b, :, 0:COLS] = res.results[2 * b]["ctx_cols"]
        ctx_out[b, :, COLS:] = res.results[2 * b + 1]["ctx_cols"]
    return out, ctx_out


# revision 19
# speedup vs baseline: 1.3036x; 1.0137x over previous
"""Bidirectional cross-attention kernel for 8 Trainium2 NeuronCores.

Sharding: core c = 2*b + g handles batch b with head-group g (8 of 16 heads).
Each core projects Q/K/V/CV for its 8 heads, computes both softmax
orientations of the shared similarity matrix, and forms the per-head
attention outputs U = attn @ cv and W = context_attn^T @ v (stored
transposed, pre-scaled by the softmax normalizers).  The two cores of a
batch exchange their U/W halves with pairwise AllGathers, after which each
core computes a disjoint 512-column slice of both final projections.

Schedule notes (v2):
 - Phase B (sim+exp+ladder) saturates the scalar engine (128 exps) while
   the PE has spare cycles; the remaining projection matmuls (QT/KT m1-3,
   V, CV) are drip-fed into the PE queue as FILLER between sim/ladder
   steps so both engines run continuously from ~8us onward, instead of a
   serial 74us projection phase with the scalar engine idle.
 - (pair, orientation, head) blocks run sequentially so only one [65,N]
   U/W accumulator is live; PSUM = 2 sim bufs + accumulator + 2 small
   projection psums = exactly 8 banks.
 - Norm path: gpsimd copies the ones-row out of PSUM, DMA roundtrips the
   broadcast, vector does reciprocal + (psum x recip) -> bf16; no
   [65,1024] staging copies.
 - Pair 3's exchanges are split (U, W-head0, W-head1) so only 8 K=64
   matmuls + adds depend on the last gather.
"""

import os
import sys
from collections import deque

import numpy as np

for _p in ("/opt/trn_rl_repo", "/root/.axon_site/_ro/trn_rl_repo"):
    if os.path.isdir(_p) and _p not in sys.path:
        sys.path.append(_p)

import ml_dtypes  # noqa: E402
import concourse.bass as bass  # noqa: E402
import concourse.mybir as mybir  # noqa: E402
import concourse.tile as tile  # noqa: E402
from concourse import bacc  # noqa: E402
from concourse.bass_utils import run_bass_kernel_spmd  # noqa: E402

B, N, DIM = 4, 1024, 1024
H, DH = 16, 64
HL = 8            # heads per core
IL = HL * DH      # local inner width (512)
COLS = 512        # output columns per core
P = 128
PAIRS = HL // 2   # head pairs per core
KCH = DIM // P    # contraction chunks (8)
ICH = N // P      # sequence chunks (8)
SCALE = DH ** -0.5
GROUPS = [[0, 1], [2, 3], [4, 5], [6, 7]]
hs = 65           # head stride in V/CV tiles (64 values + ones column)

F32 = mybir.dt.float32
BF16 = mybir.dt.bfloat16
EXP = mybir.ActivationFunctionType.Exp

_CACHED_NC = None


def _build_nc():
    nc = bacc.Bacc("TRN2", target_bir_lowering=False, debug=False, num_devices=8)

    T = {}
    for nm, shape, dt in (
            ("xT", [DIM, N], BF16), ("ctxT", [DIM, N], BF16),
            ("wqk", [DIM, IL], BF16), ("wv", [DIM, IL], BF16),
            ("cwqk", [DIM, IL], BF16), ("cwv", [DIM, IL], BF16),
            ("wout", [DIM, COLS], BF16), ("cwout", [DIM, COLS], BF16),
            ("bout", [1, COLS], F32), ("cbout", [1, COLS], F32)):
        T[nm] = nc.dram_tensor(nm, shape, dt, kind="ExternalInput")
    T["out_cols"] = nc.dram_tensor("out_cols", [N, COLS], F32, kind="ExternalOutput")
    T["ctx_cols"] = nc.dram_tensor("ctx_cols", [N, COLS], F32, kind="ExternalOutput")

    with tile.TileContext(nc) as tc:
        with tc.tile_pool(name="dram", bufs=1, space="DRAM") as dpool:
            T["uwl"] = [dpool.tile([256, N], BF16, tag=f"uwl{p}", name=f"uwl{p}")
                        for p in range(4)]
            T["uwa"] = [dpool.tile([512, N], BF16, tag=f"uwa{p}", name=f"uwa{p}")
                        for p in range(3)]
            T["uwa3u"] = dpool.tile([256, N], BF16, tag="uwa3u", name="uwa3u")
            T["uwa3wh"] = [dpool.tile([128, N], BF16, tag=f"uwa3wh{h}",
                                      name=f"uwa3wh{h}")
                           for h in range(2)]
            T["normd"] = dpool.tile([16, N], F32, tag="normd", name="normd")
            _build_body(nc, tc, T)
    nc.compile()
    if os.environ.get("KERNEL_LDW_DEDUP", "1") == "1":
        _dedupe_ldweights(nc)
    return nc


def _dedupe_ldweights(nc):
    """Drop PE Ldweights that reload the exact weights already resident."""
    def sig(i):
        a = i.ins[0]
        return (a.memref, a.offset, str(a.ap), str(a.dtype),
                str(i.tile_position), str(i.tile_size),
                str(i.perf_mode), str(i.is_transpose))

    removed = 0
    for fn in nc.m.functions:
        for bb in fn.blocks:
            last = None
            keep = []
            for i in bb.instructions:
                if isinstance(i, mybir.InstLdweights):
                    s = sig(i)
                    si = i.sync_info
                    if s == last and (si is None or
                                      (not si.on_wait and not si.on_update)):
                        removed += 1
                        continue
                    last = s
                elif isinstance(i, mybir.InstMatmult):
                    pass
                elif getattr(i, "engine", None) == mybir.EngineType.PE:
                    last = None
                keep.append(i)
            if removed:
                bb.instructions = keep
    return removed


def _build_body(nc, tc, T):
    from contextlib import ExitStack
    stack = ExitStack()       # pools that live to the end
    bstack = ExitStack()      # PSUM pool released before the finals
    iostack = ExitStack()     # input pools released after the projections
    pqk = stack.enter_context(tc.tile_pool(name="pqk", bufs=1))
    pv = stack.enter_context(tc.tile_pool(name="pv", bufs=1))
    pf = stack.enter_context(tc.tile_pool(name="pf", bufs=1))
    pu = stack.enter_context(tc.tile_pool(name="pu", bufs=1))
    pe = stack.enter_context(tc.tile_pool(name="pe", bufs=4))
    pn = stack.enter_context(tc.tile_pool(name="pn", bufs=2))
    pn1 = stack.enter_context(tc.tile_pool(name="pn1", bufs=1))
    psB = bstack.enter_context(tc.tile_pool(name="psB", bufs=1, space="PSUM"))
    pw = iostack.enter_context(tc.tile_pool(name="pw", bufs=1))
    pin = iostack.enter_context(tc.tile_pool(name="pin", bufs=1))

    # ---------------- input DMA ----------------
    # critical path: wqk + xT (sync queue) and cwqk + ctxT (scalar queue,
    # which is idle until phase B starts).
    wqk_t, xt, cwqk_t, ct = [], [], [], []
    for k in range(KCH):
        w = pw.tile([P, IL], BF16, tag=f"wqk{k}", name=f"wqk{k}")
        nc.gpsimd.dma_start(w[:], T["wqk"][k * P:(k + 1) * P, :])
        wqk_t.append(w)
        t = pin.tile([P, N], BF16, tag=f"xT{k}", name=f"xt{k}")
        nc.sync.dma_start(t[:], T["xT"][k * P:(k + 1) * P, :])
        xt.append(t)
        w = pw.tile([P, IL], BF16, tag=f"cwqk{k}", name=f"cwqk{k}")
        nc.scalar.dma_start(w[:], T["cwqk"][k * P:(k + 1) * P, :])
        cwqk_t.append(w)
    for k in range(KCH):
        t = pin.tile([P, N], BF16, tag=f"cT{k}", name=f"ct{k}")
        nc.scalar.dma_start(t[:], T["ctxT"][k * P:(k + 1) * P, :])
        ct.append(t)
    wv_t, cwv_t = [], []
    for k in range(KCH):
        w = pw.tile([P, IL], BF16, tag=f"cwv{k}", name=f"cwv{k}")
        nc.gpsimd.dma_start(w[:], T["cwv"][k * P:(k + 1) * P, :])
        cwv_t.append(w)
    for k in range(KCH):
        w = pw.tile([P, IL], BF16, tag=f"wv{k}", name=f"wv{k}")
        nc.gpsimd.dma_start(w[:], T["wv"][k * P:(k + 1) * P, :])
        wv_t.append(w)
    # output-side weights/biases (needed in phase C only)
    bout_bc = pf.tile([P, COLS], F32, tag="bb")
    nc.gpsimd.dma_start(bout_bc[:], T["bout"][:].to_broadcast((P, COLS)))
    cbout_bc = pf.tile([P, COLS], F32, tag="cbb")
    nc.gpsimd.dma_start(cbout_bc[:], T["cbout"][:].to_broadcast((P, COLS)))
    wout_sb, cwout_sb = [], []
    for k in range(KCH):
        t = pf.tile([P, COLS], BF16, tag=f"wo{k}")
        nc.gpsimd.dma_start(t[:], T["wout"][k * P:(k + 1) * P, :])
        wout_sb.append(t)
        t = pf.tile([P, COLS], BF16, tag=f"cwo{k}")
        nc.gpsimd.dma_start(t[:], T["cwout"][k * P:(k + 1) * P, :])
        cwout_sb.append(t)

    # ---------------- projection emitters (filler items) ----------------
    QT = [None] * PAIRS   # QT[m] = (pa, pb): head A rows 0:64 / head B 64:128
    KT = [None] * PAIRS
    V = [None] * ICH      # [128, HL*hs] bf16, ones col per head
    CV = [None] * ICH

    def projT_items(src, wtiles, store, m, tag):
        pa = pqk.tile([P, N], BF16, tag=f"{tag}a{m}")
        pb = pqk.tile([P, N], BF16, tag=f"{tag}b{m}")
        store[m] = (pa, pb)
        items = [lambda: (nc.vector.memset(pa[DH:P, :], 0.0),
                          nc.vector.memset(pb[0:DH, :], 0.0))]
        for half in range(2):
            ps = psB.tile([P, COLS], F32, tag="pt", name=f"pt_{tag}{m}_{half}")
            lo = half * COLS

            def mk(k, ps=ps, lo=lo):
                def it():
                    nc.tensor.matmul(ps[:], wtiles[k][:, m * P:(m + 1) * P],
                                     src[k][:, lo:lo + COLS],
                                     start=(k == 0), stop=(k == KCH - 1))
                    if k == KCH - 1:
                        nc.vector.tensor_copy(pa[0:DH, lo:lo + COLS],
                                              ps[0:DH, :])
                        nc.vector.tensor_copy(pb[DH:P, lo:lo + COLS],
                                              ps[DH:P, :])
                return it
            items.extend(mk(k) for k in range(KCH))
        return items

    def projV_items(src, wtiles, store, ic, tag):
        o = pv.tile([P, HL * hs], BF16, tag=f"{tag}{ic}")
        store[ic] = o
        ps = psB.tile([P, IL], F32, tag="pv", name=f"pv_{tag}{ic}")
        items = []

        def mk(k):
            def it():
                nc.tensor.matmul(ps[:], src[k][:, ic * P:(ic + 1) * P],
                                 wtiles[k][:],
                                 start=(k == 0), stop=(k == KCH - 1))
                if k == KCH - 1:
                    dst = o[:].rearrange("p (h e) -> p h e", e=hs)
                    nc.vector.tensor_copy(
                        dst[:, :, 0:DH],
                        ps[:].rearrange("p (h e) -> p h e", e=DH))
                    nc.vector.memset(dst[:, :, DH:hs], 1.0)
            return it
        items.extend(mk(k) for k in range(KCH))
        return items

    # resource name -> remaining items; drip order for background filling
    res = {}
    for ic in range(ICH):
        res[f"cv{ic}"] = projV_items(ct, cwv_t, CV, ic, "cv")
    for ic in range(ICH):
        res[f"v{ic}"] = projV_items(xt, wv_t, V, ic, "v")
    for m in (1, 2, 3):
        res[f"kt{m}"] = projT_items(ct, cwqk_t, KT, m, "kt")
        res[f"qt{m}"] = projT_items(xt, wqk_t, QT, m, "qt")
    drip = deque(
        [f"cv{ic}" for ic in range(ICH)] + ["kt1", "qt1"] +
        [f"v{ic}" for ic in range(ICH)] +
        ["kt2", "qt2", "kt3", "qt3"])

    def require(name):
        for it in res.pop(name, ()):
            it()

    def emit_fillers(n):
        done = 0
        while done < n and drip:
            lst = res.get(drip[0])
            if not lst:
                res.pop(drip[0], None)
                drip.popleft()
                continue
            lst.pop(0)()
            done += 1

    def drain_fillers():
        while drip:
            emit_fillers(len(drip) * 32)

    # ---------------- prelude: QT/KT m0 ----------------
    for it in projT_items(xt, wqk_t, QT, 0, "qt"):
        it()
    for it in projT_items(ct, cwqk_t, KT, 0, "kt"):
        it()

    # ---------------- phase B ----------------
    u_sb = [None] * KCH
    w_sb = [None] * KCH

    def norm(p, orient, hh, acc):
        """acc: [65,N] psum; rows 0:64 data, row 64 normalizer."""
        slot = 4 * p + 2 * orient + hh
        rst = pn1.tile([DH + 1, N], F32, tag="rst")
        nc.vector.tensor_copy(rst[:], acc[0:DH + 1, :])
        nc.sync.dma_start(T["normd"][slot:slot + 1, :], rst[DH:DH + 1, :])
        rbc = pn.tile([DH, N], F32, tag="rbc")
        nc.sync.dma_start(
            rbc[:], T["normd"][slot:slot + 1, :].to_broadcast((DH, N)))
        nc.vector.reciprocal_approx_fast(rbc[:], rbc[:])
        ubf = pn.tile([DH, N], BF16, tag="ubf")
        nc.vector.tensor_mul(ubf[:], rst[0:DH, :], rbc[:])
        r0 = orient * 128 + hh * DH
        nc.sync.dma_start(T["uwl"][p][r0:r0 + DH, :], ubf[:])

    def block(p, orient, hh):
        """One (pair, orientation, head) unit: 8 sim+exp steps with the
        U/W ladder at lag 2 and fillers padding the PE."""
        h = 2 * p + hh
        require(f"qt{p}")
        require(f"kt{p}")
        if orient == 0:   # U: simT (j on partitions), ladder vs CV
            lhsT_full, rhs_full = KT[p][hh], QT[p][hh]
            lad = CV
        else:             # W: sim (i on partitions), ladder vs V
            lhsT_full, rhs_full = QT[p][hh], KT[p][hh]
            lad = V
        acc = psB.tile([P, N], F32, tag="uw", name=f"uw{p}_{orient}_{hh}")
        E = [None] * ICH

        def ladder(jc):
            require(f"{'cv' if orient == 0 else 'v'}{jc}")
            lt = lad[jc][:, h * hs:(h + 1) * hs]
            for half in range(2):
                nc.tensor.matmul(
                    acc[0:hs, half * COLS:(half + 1) * COLS],
                    lt, E[jc][:, half * COLS:(half + 1) * COLS],
                    start=(jc == 0), stop=(jc == ICH - 1))

        for jc in range(ICH):
            ps = psB.tile([P, N], F32, tag=f"sim{jc & 1}")
            lhsT = lhsT_full[:, jc * P:(jc + 1) * P]
            for half in range(2):
                nc.tensor.matmul(ps[:, half * COLS:(half + 1) * COLS],
                                 lhsT, rhs_full[:, half * COLS:(half + 1) * COLS],
                                 start=True, stop=True)
            e = pe.tile([P, N], BF16, tag="E")
            nc.scalar.activation(e[:], ps[:], EXP, scale=SCALE)
            E[jc] = e
            if jc >= 2:
                ladder(jc - 2)
            emit_fillers(2)
        ladder(ICH - 2)
        ladder(ICH - 1)
        norm(p, orient, hh, acc)

    def load_pair(p):
        rows = ((0, u_sb, "u", 2 * p), (256, u_sb, "u", 2 * p + 1),
                (128, w_sb, "w", 2 * p), (384, w_sb, "w", 2 * p + 1))
        for row, arr, nm, k in rows:
            t = pu.tile([P, N], BF16, tag=f"{nm}sb{k}", name=f"{nm}sb{k}")
            nc.sync.dma_start(t[:], T["uwa"][p][row:row + P, :])
            arr[k] = t

    # partial output projections (k chunks 0..5, from pairs 0-2) run as
    # pair-3 fillers in the pt/pv psum slots freed by the projections.
    cp_out = [None] * ICH
    cp_ctx = [None] * ICH
    pc = [None]

    def partial_items(ic):
        pso = psB.tile([P, COLS], F32, tag="pt", name=f"pso{ic}")
        cpo = pc[0].tile([P, COLS], F32, tag=f"ocp{ic}", name=f"ocp{ic}")
        cp_out[ic] = cpo
        psc = psB.tile([P, IL], F32, tag="pv", name=f"psc{ic}")
        cpc = pc[0].tile([P, COLS], F32, tag=f"ccp{ic}", name=f"ccp{ic}")
        cp_ctx[ic] = cpc

        def mko(k):
            def it():
                nc.tensor.matmul(pso[:], u_sb[k][:, ic * P:(ic + 1) * P],
                                 wout_sb[k][:], start=(k == 0), stop=(k == 5))
                if k == 5:
                    nc.vector.tensor_add(cpo[:], pso[:], bout_bc[:])
            return it

        def mkc(k):
            def it():
                nc.tensor.matmul(psc[:], w_sb[k][:, ic * P:(ic + 1) * P],
                                 cwout_sb[k][:], start=(k == 0), stop=(k == 5))
                if k == 5:
                    nc.vector.tensor_add(cpc[:], psc[:], cbout_bc[:])
            return it
        return ([mko(k) for k in range(6)], [mkc(k) for k in range(6)])

    for p in range(PAIRS):
        block(p, 0, 0)
        block(p, 0, 1)
        if p == 3:
            nc.gpsimd.collective_compute(
                "AllGather", mybir.AluOpType.bypass,
                replica_groups=GROUPS,
                ins=[T["uwl"][3][0:128, :]],
                outs=[T["uwa3u"][:]],
            )
            for j in range(2):
                t = pu.tile([P, N], BF16, tag=f"usb{6 + j}", name=f"usb{6 + j}")
                nc.sync.dma_start(t[:], T["uwa3u"][j * P:(j + 1) * P, :])
                u_sb[6 + j] = t
        block(p, 1, 0)
        if p == 3:
            nc.gpsimd.collective_compute(
                "AllGather", mybir.AluOpType.bypass,
                replica_groups=GROUPS,
                ins=[T["uwl"][3][128:128 + DH, :]],
                outs=[T["uwa3wh"][0][:]],
            )
        block(p, 1, 1)
        if p < 3:
            nc.gpsimd.collective_compute(
                "AllGather", mybir.AluOpType.bypass,
                replica_groups=GROUPS,
                ins=[T["uwl"][p][:]],
                outs=[T["uwa"][p][:]],
            )
            load_pair(p)
            if p == 2:
                # projections are done by now; free their input pools and
                # queue the partial output projections as pair-3 fillers.
                drain_fillers()
                iostack.close()
                pc[0] = stack.enter_context(tc.tile_pool(name="pc", bufs=1))
                for ic in range(ICH):
                    oi, ci = partial_items(ic)
                    res[f"op{ic}"] = oi
                    res[f"cp{ic}"] = ci
                    drip.append(f"op{ic}")
                    drip.append(f"cp{ic}")
        else:
            nc.gpsimd.collective_compute(
                "AllGather", mybir.AluOpType.bypass,
                replica_groups=GROUPS,
                ins=[T["uwl"][3][128 + DH:256, :]],
                outs=[T["uwa3wh"][1][:]],
            )
            w6 = pu.tile([P, N], BF16, tag="wsb6")
            w7 = pu.tile([P, N], BF16, tag="wsb7")
            for hh in range(2):
                nc.sync.dma_start(w6[hh * DH:(hh + 1) * DH, :],
                                  T["uwa3wh"][hh][0:DH, :])
                nc.sync.dma_start(w7[hh * DH:(hh + 1) * DH, :],
                                  T["uwa3wh"][hh][DH:P, :])
            w_sb[6], w_sb[7] = w6, w7

    drain_fillers()
    bstack.close()   # free psB PSUM for the finals

    # ---------------- phase C: gather-dependent finals only ----------------
    with tc.tile_pool(name="po", bufs=6) as po, \
         tc.tile_pool(name="psD", bufs=4, space="PSUM") as psD:
        for ic in range(ICH):
            ps = psD.tile([P, COLS], F32, tag="od", name=f"outf{ic}")
            for k in (6, 7):
                nc.tensor.matmul(ps[:], u_sb[k][:, ic * P:(ic + 1) * P],
                                 wout_sb[k][:], start=(k == 6), stop=(k == 7))
            o = po.tile([P, COLS], F32, tag="ot")
            nc.vector.tensor_add(o[:], ps[:], cp_out[ic][:])
            (nc.sync if ic % 2 else nc.scalar).dma_start(
                T["out_cols"][ic * P:(ic + 1) * P, :], o[:])
        for ic in range(ICH):
            ps = psD.tile([P, COLS], F32, tag="od", name=f"ctxf{ic}")
            for j, (k, r0) in enumerate(
                    ((6, 0), (7, 0), (6, DH), (7, DH))):
                nc.tensor.matmul(ps[:], w_sb[k][r0:r0 + DH, ic * P:(ic + 1) * P],
                                 cwout_sb[k][r0:r0 + DH, :],
                                 start=(j == 0), stop=(j == 3))
            o = po.tile([P, COLS], F32, tag="ot")
            nc.vector.tensor_add(o[:], ps[:], cp_ctx[ic][:])
            (nc.sync if ic % 2 else nc.scalar).dma_start(
                T["ctx_cols"][ic * P:(ic + 1) * P, :], o[:])
    stack.close()


def _get_nc():
    global _CACHED_NC
    if _CACHED_NC is None:
        _CACHED_NC = _build_nc()
    return _CACHED_NC


def _reorder_rows(w):
    """Reorder [INNER, :] rows to the uw_all K-chunk order (p-major, group X)."""
    chunks = []
    for p in range(4):
        for X in range(2):
            chunks.append(w[X * 512 + p * 128:X * 512 + (p + 1) * 128])
    return np.concatenate(chunks, axis=0)


def kernel(x, context, w_qk, w_v, cw_qk, cw_v, w_out, b_out, cw_out, cb_out):
    x = np.asarray(x, dtype=np.float32)
    context = np.asarray(context, dtype=np.float32)
    w_qk = np.asarray(w_qk, dtype=np.float32)
    w_v = np.asarray(w_v, dtype=np.float32)
    cw_qk = np.asarray(cw_qk, dtype=np.float32)
    cw_v = np.asarray(cw_v, dtype=np.float32)
    w_out_r = _reorder_rows(np.asarray(w_out, dtype=np.float32)).astype(ml_dtypes.bfloat16)
    cw_out_r = _reorder_rows(np.asarray(cw_out, dtype=np.float32)).astype(ml_dtypes.bfloat16)
    b_out = np.asarray(b_out, dtype=np.float32)
    cb_out = np.asarray(cb_out, dtype=np.float32)

    in_maps = []
    for c in range(8):
        b, g = c // 2, c % 2
        sl = slice(g * IL, (g + 1) * IL)
        in_maps.append({
            "xT": np.ascontiguousarray(x[b].T).astype(ml_dtypes.bfloat16),
            "ctxT": np.ascontiguousarray(context[b].T).astype(ml_dtypes.bfloat16),
            "wqk": np.ascontiguousarray(w_qk[:, sl]).astype(ml_dtypes.bfloat16),
            "wv": np.ascontiguousarray(w_v[:, sl]).astype(ml_dtypes.bfloat16),
            "cwqk": np.ascontiguousarray(cw_qk[:, sl]).astype(ml_dtypes.bfloat16),
            "cwv": np.ascontiguousarray(cw_v[:, sl]).astype(ml_dtypes.bfloat16),
            "wout": np.ascontiguousarray(w_out_r[:, sl]),
            "cwout": np.ascontiguousarray(cw_out_r[:, sl]),
            "bout": np.ascontiguousarray(b_out[None, sl]),
            "cbout": np.ascontiguousarray(cb_out[None, sl]),
        })

    nc = _get_nc()
    res = run_bass_kernel_spmd(nc, in_maps, list(range(8)))

    out = np.empty((B, N, DIM), dtype=np.float32)
    ctx_out = np.empty((B, N, DIM), dtype=np.float32)
    for b in range(B):
        out[b, :, 0:COLS] = res.results[2 * b]["out_cols"]
        out[b, :, COLS:] = res.results[2 * b + 1]["out_cols"]
        ctx_out[b, :, 0:COLS] = res.results[2 * b]["ctx_cols"]
        ctx_out[b, :, COLS:] = res.results[2 * b + 1]["ctx_cols"]
    return out, ctx_out


# revision 22
# speedup vs baseline: 1.3055x; 1.0014x over previous
"""Bidirectional cross-attention kernel for 8 Trainium2 NeuronCores.

Sharding: core c = 2*b + g handles batch b with head-group g (8 of 16 heads).
Each core projects Q/K/V/CV for its 8 heads, computes both softmax
orientations of the shared similarity matrix, and forms the per-head
attention outputs U = attn @ cv and W = context_attn^T @ v (stored
transposed, pre-scaled by the softmax normalizers).  The two cores of a
batch exchange their U/W halves with pairwise AllGathers, after which each
core computes a disjoint 512-column slice of both final projections.

Schedule notes (v2):
 - Phase B (sim+exp+ladder) saturates the scalar engine (128 exps) while
   the PE has spare cycles; the remaining projection matmuls (QT/KT m1-3,
   V, CV) are drip-fed into the PE queue as FILLER between sim/ladder
   steps so both engines run continuously from ~8us onward, instead of a
   serial 74us projection phase with the scalar engine idle.
 - (pair, orientation, head) blocks run sequentially so only one [65,N]
   U/W accumulator is live; PSUM = 2 sim bufs + accumulator + 2 small
   projection psums = exactly 8 banks.
 - Norm path: gpsimd copies the ones-row out of PSUM, DMA roundtrips the
   broadcast, vector does reciprocal + (psum x recip) -> bf16; no
   [65,1024] staging copies.
 - Pair 3's exchanges are split (U, W-head0, W-head1) so only 8 K=64
   matmuls + adds depend on the last gather.
"""

import os
import sys
from collections import deque

import numpy as np

for _p in ("/opt/trn_rl_repo", "/root/.axon_site/_ro/trn_rl_repo"):
    if os.path.isdir(_p) and _p not in sys.path:
        sys.path.append(_p)

import ml_dtypes  # noqa: E402
import concourse.bass as bass  # noqa: E402
import concourse.mybir as mybir  # noqa: E402
import concourse.tile as tile  # noqa: E402
from concourse import bacc  # noqa: E402
from concourse.bass_utils import run_bass_kernel_spmd  # noqa: E402

B, N, DIM = 4, 1024, 1024
H, DH = 16, 64
HL = 8            # heads per core
IL = HL * DH      # local inner width (512)
COLS = 512        # output columns per core
P = 128
PAIRS = HL // 2   # head pairs per core
KCH = DIM // P    # contraction chunks (8)
ICH = N // P      # sequence chunks (8)
SCALE = DH ** -0.5
GROUPS = [[0, 1], [2, 3], [4, 5], [6, 7]]
hs = 65           # head stride in V/CV tiles (64 values + ones column)

F32 = mybir.dt.float32
BF16 = mybir.dt.bfloat16
EXP = mybir.ActivationFunctionType.Exp

_CACHED_NC = None


def _build_nc():
    nc = bacc.Bacc("TRN2", target_bir_lowering=False, debug=False, num_devices=8)

    T = {}
    for nm, shape, dt in (
            ("xT", [DIM, N], BF16), ("ctxT", [DIM, N], BF16),
            ("wqk", [DIM, IL], BF16), ("wv", [DIM, IL], BF16),
            ("cwqk", [DIM, IL], BF16), ("cwv", [DIM, IL], BF16),
            ("wout", [DIM, COLS], BF16), ("cwout", [DIM, COLS], BF16),
            ("bout", [1, COLS], F32), ("cbout", [1, COLS], F32)):
        T[nm] = nc.dram_tensor(nm, shape, dt, kind="ExternalInput")
    T["out_cols"] = nc.dram_tensor("out_cols", [N, COLS], F32, kind="ExternalOutput")
    T["ctx_cols"] = nc.dram_tensor("ctx_cols", [N, COLS], F32, kind="ExternalOutput")

    with tile.TileContext(nc) as tc:
        with tc.tile_pool(name="dram", bufs=1, space="DRAM") as dpool:
            T["uwl"] = [dpool.tile([256, N], BF16, tag=f"uwl{p}", name=f"uwl{p}")
                        for p in range(4)]
            T["uwa"] = [dpool.tile([512, N], BF16, tag=f"uwa{p}", name=f"uwa{p}")
                        for p in range(3)]
            T["uwa3u"] = dpool.tile([256, N], BF16, tag="uwa3u", name="uwa3u")
            T["uwa3wh"] = [dpool.tile([128, N], BF16, tag=f"uwa3wh{h}",
                                      name=f"uwa3wh{h}")
                           for h in range(2)]
            T["normd"] = dpool.tile([16, N], F32, tag="normd", name="normd")
            _build_body(nc, tc, T)
    nc.compile()
    if os.environ.get("KERNEL_LDW_DEDUP", "1") == "1":
        _dedupe_ldweights(nc)
    return nc


def _dedupe_ldweights(nc):
    """Drop PE Ldweights that reload the exact weights already resident."""
    def sig(i):
        a = i.ins[0]
        return (a.memref, a.offset, str(a.ap), str(a.dtype),
                str(i.tile_position), str(i.tile_size),
                str(i.perf_mode), str(i.is_transpose))

    removed = 0
    for fn in nc.m.functions:
        for bb in fn.blocks:
            last = None
            keep = []
            for i in bb.instructions:
                if isinstance(i, mybir.InstLdweights):
                    s = sig(i)
                    si = i.sync_info
                    if s == last and (si is None or
                                      (not si.on_wait and not si.on_update)):
                        removed += 1
                        continue
                    last = s
                elif isinstance(i, mybir.InstMatmult):
                    pass
                elif getattr(i, "engine", None) == mybir.EngineType.PE:
                    last = None
                keep.append(i)
            if removed:
                bb.instructions = keep
    return removed


def _build_body(nc, tc, T):
    from contextlib import ExitStack
    stack = ExitStack()       # pools that live to the end
    bstack = ExitStack()      # PSUM pool released before the finals
    iostack = ExitStack()     # input pools released after the projections
    pqk = stack.enter_context(tc.tile_pool(name="pqk", bufs=1))
    pv = stack.enter_context(tc.tile_pool(name="pv", bufs=1))
    pf = stack.enter_context(tc.tile_pool(name="pf", bufs=1))
    pu = stack.enter_context(tc.tile_pool(name="pu", bufs=1))
    pe = stack.enter_context(tc.tile_pool(name="pe", bufs=4))
    pn = stack.enter_context(tc.tile_pool(name="pn", bufs=2))
    pn1 = stack.enter_context(tc.tile_pool(name="pn1", bufs=1))
    psB = bstack.enter_context(tc.tile_pool(name="psB", bufs=1, space="PSUM"))
    pw = iostack.enter_context(tc.tile_pool(name="pw", bufs=1))
    pin = iostack.enter_context(tc.tile_pool(name="pin", bufs=1))

    # ---------------- input DMA ----------------
    # critical path: wqk + xT (sync queue) and cwqk + ctxT (scalar queue,
    # which is idle until phase B starts).
    wqk_t, xt, cwqk_t, ct = [], [], [], []
    for k in range(KCH):
        w = pw.tile([P, IL], BF16, tag=f"wqk{k}", name=f"wqk{k}")
        nc.gpsimd.dma_start(w[:], T["wqk"][k * P:(k + 1) * P, :])
        wqk_t.append(w)
        t = pin.tile([P, N], BF16, tag=f"xT{k}", name=f"xt{k}")
        nc.sync.dma_start(t[:], T["xT"][k * P:(k + 1) * P, :])
        xt.append(t)
        w = pw.tile([P, IL], BF16, tag=f"cwqk{k}", name=f"cwqk{k}")
        nc.scalar.dma_start(w[:], T["cwqk"][k * P:(k + 1) * P, :])
        cwqk_t.append(w)
    for k in range(KCH):
        t = pin.tile([P, N], BF16, tag=f"cT{k}", name=f"ct{k}")
        (nc.sync if k % 2 else nc.scalar).dma_start(
            t[:], T["ctxT"][k * P:(k + 1) * P, :])
        ct.append(t)
    wv_t, cwv_t = [], []
    for k in range(KCH):
        w = pw.tile([P, IL], BF16, tag=f"cwv{k}", name=f"cwv{k}")
        nc.gpsimd.dma_start(w[:], T["cwv"][k * P:(k + 1) * P, :])
        cwv_t.append(w)
    for k in range(KCH):
        w = pw.tile([P, IL], BF16, tag=f"wv{k}", name=f"wv{k}")
        nc.gpsimd.dma_start(w[:], T["wv"][k * P:(k + 1) * P, :])
        wv_t.append(w)
    # output-side weights/biases (needed in phase C only)
    bout_bc = pf.tile([P, COLS], F32, tag="bb")
    nc.gpsimd.dma_start(bout_bc[:], T["bout"][:].to_broadcast((P, COLS)))
    cbout_bc = pf.tile([P, COLS], F32, tag="cbb")
    nc.gpsimd.dma_start(cbout_bc[:], T["cbout"][:].to_broadcast((P, COLS)))
    wout_sb, cwout_sb = [], []
    for k in range(KCH):
        t = pf.tile([P, COLS], BF16, tag=f"wo{k}")
        nc.gpsimd.dma_start(t[:], T["wout"][k * P:(k + 1) * P, :])
        wout_sb.append(t)
        t = pf.tile([P, COLS], BF16, tag=f"cwo{k}")
        nc.gpsimd.dma_start(t[:], T["cwout"][k * P:(k + 1) * P, :])
        cwout_sb.append(t)

    # ---------------- projection emitters (filler items) ----------------
    QT = [None] * PAIRS   # QT[m] = (pa, pb): head A rows 0:64 / head B 64:128
    KT = [None] * PAIRS
    V = [None] * ICH      # [128, HL*hs] bf16, ones col per head
    CV = [None] * ICH

    def projT_items(src, wtiles, store, m, tag):
        pa = pqk.tile([P, N], BF16, tag=f"{tag}a{m}")
        pb = pqk.tile([P, N], BF16, tag=f"{tag}b{m}")
        store[m] = (pa, pb)
        items = [lambda: (nc.vector.memset(pa[DH:P, :], 0.0),
                          nc.vector.memset(pb[0:DH, :], 0.0))]
        for half in range(2):
            ps = psB.tile([P, COLS], F32, tag="pt", name=f"pt_{tag}{m}_{half}")
            lo = half * COLS

            def mk(k, ps=ps, lo=lo):
                def it():
                    nc.tensor.matmul(ps[:], wtiles[k][:, m * P:(m + 1) * P],
                                     src[k][:, lo:lo + COLS],
                                     start=(k == 0), stop=(k == KCH - 1))
                    if k == KCH - 1:
                        nc.vector.tensor_copy(pa[0:DH, lo:lo + COLS],
                                              ps[0:DH, :])
                        nc.vector.tensor_copy(pb[DH:P, lo:lo + COLS],
                                              ps[DH:P, :])
                return it
            items.extend(mk(k) for k in range(KCH))
        return items

    def projV_items(src, wtiles, store, ic, tag):
        o = pv.tile([P, HL * hs], BF16, tag=f"{tag}{ic}")
        store[ic] = o
        ps = psB.tile([P, IL], F32, tag="pv", name=f"pv_{tag}{ic}")
        items = []

        def mk(k):
            def it():
                nc.tensor.matmul(ps[:], src[k][:, ic * P:(ic + 1) * P],
                                 wtiles[k][:],
                                 start=(k == 0), stop=(k == KCH - 1))
                if k == KCH - 1:
                    dst = o[:].rearrange("p (h e) -> p h e", e=hs)
                    nc.vector.tensor_copy(
                        dst[:, :, 0:DH],
                        ps[:].rearrange("p (h e) -> p h e", e=DH))
                    nc.vector.memset(dst[:, :, DH:hs], 1.0)
            return it
        items.extend(mk(k) for k in range(KCH))
        return items

    # resource name -> remaining items; drip order for background filling
    res = {}
    for ic in range(ICH):
        res[f"cv{ic}"] = projV_items(ct, cwv_t, CV, ic, "cv")
    for ic in range(ICH):
        res[f"v{ic}"] = projV_items(xt, wv_t, V, ic, "v")
    for m in (1, 2, 3):
        res[f"kt{m}"] = projT_items(ct, cwqk_t, KT, m, "kt")
        res[f"qt{m}"] = projT_items(xt, wqk_t, QT, m, "qt")
    drip = deque(
        [f"cv{ic}" for ic in range(ICH)] + ["kt1", "qt1"] +
        [f"v{ic}" for ic in range(ICH)] +
        ["kt2", "qt2", "kt3", "qt3"])

    def require(name):
        for it in res.pop(name, ()):
            it()

    def emit_fillers(n):
        done = 0
        while done < n and drip:
            lst = res.get(drip[0])
            if not lst:
                res.pop(drip[0], None)
                drip.popleft()
                continue
            lst.pop(0)()
            done += 1

    def drain_fillers():
        while drip:
            emit_fillers(len(drip) * 32)

    # ---------------- prelude: QT/KT m0 ----------------
    for it in projT_items(xt, wqk_t, QT, 0, "qt"):
        it()
    for it in projT_items(ct, cwqk_t, KT, 0, "kt"):
        it()

    # ---------------- phase B ----------------
    u_sb = [None] * KCH
    w_sb = [None] * KCH

    def norm(p, orient, hh, acc):
        """acc: [65,N] psum; rows 0:64 data, row 64 normalizer."""
        slot = 4 * p + 2 * orient + hh
        rst = pn1.tile([DH + 1, N], F32, tag="rst")
        nc.vector.tensor_copy(rst[:], acc[0:DH + 1, :])
        nc.sync.dma_start(T["normd"][slot:slot + 1, :], rst[DH:DH + 1, :])
        rbc = pn.tile([DH, N], F32, tag="rbc")
        nc.sync.dma_start(
            rbc[:], T["normd"][slot:slot + 1, :].to_broadcast((DH, N)))
        nc.vector.reciprocal_approx_fast(rbc[:], rbc[:])
        ubf = pn.tile([DH, N], BF16, tag="ubf")
        nc.vector.tensor_mul(ubf[:], rst[0:DH, :], rbc[:])
        r0 = orient * 128 + hh * DH
        nc.sync.dma_start(T["uwl"][p][r0:r0 + DH, :], ubf[:])

    def block(p, orient, hh, pending, nfill=2):
        """One (pair, orientation, head) unit: 8 sim+exp steps with the
        U/W ladder at lag 2 and fillers padding the PE.  The tail
        (last two ladder steps + norm) is returned as a closure and runs
        inside the NEXT block's first step, so the exp(7) -> ladder(7)
        chain never delays the next block's first sim/exp."""
        h = 2 * p + hh
        require(f"qt{p}")
        require(f"kt{p}")
        if orient == 0:   # U: simT (j on partitions), ladder vs CV
            lhsT_full, rhs_full = KT[p][hh], QT[p][hh]
            lad = CV
        else:             # W: sim (i on partitions), ladder vs V
            lhsT_full, rhs_full = QT[p][hh], KT[p][hh]
            lad = V
        acc = psB.tile([P, N], F32, tag="uw", name=f"uw{p}_{orient}_{hh}")
        E = [None] * ICH

        def ladder(jc):
            require(f"{'cv' if orient == 0 else 'v'}{jc}")
            lt = lad[jc][:, h * hs:(h + 1) * hs]
            for half in range(2):
                nc.tensor.matmul(
                    acc[0:hs, half * COLS:(half + 1) * COLS],
                    lt, E[jc][:, half * COLS:(half + 1) * COLS],
                    start=(jc == 0), stop=(jc == ICH - 1))

        for jc in range(ICH):
            ps = psB.tile([P, N], F32, tag=f"sim{jc & 1}")
            lhsT = lhsT_full[:, jc * P:(jc + 1) * P]
            for half in range(2):
                nc.tensor.matmul(ps[:, half * COLS:(half + 1) * COLS],
                                 lhsT, rhs_full[:, half * COLS:(half + 1) * COLS],
                                 start=True, stop=True)
            e = pe.tile([P, N], BF16, tag="E")
            nc.scalar.activation(e[:], ps[:], EXP, scale=SCALE)
            E[jc] = e
            if jc == 0 and pending is not None:
                pending()
            if jc >= 2:
                ladder(jc - 2)
            emit_fillers(nfill)

        def tail():
            ladder(ICH - 2)
            ladder(ICH - 1)
            norm(p, orient, hh, acc)
        return tail

    def load_pair(p):
        rows = ((0, u_sb, "u", 2 * p), (256, u_sb, "u", 2 * p + 1),
                (128, w_sb, "w", 2 * p), (384, w_sb, "w", 2 * p + 1))
        for row, arr, nm, k in rows:
            t = pu.tile([P, N], BF16, tag=f"{nm}sb{k}", name=f"{nm}sb{k}")
            nc.sync.dma_start(t[:], T["uwa"][p][row:row + P, :])
            arr[k] = t

    # partial output projections (k chunks 0..5, from pairs 0-2) run as
    # pair-3 fillers in the pt/pv psum slots freed by the projections.
    cp_out = [None] * ICH
    cp_ctx = [None] * ICH
    pc = [None]

    def partial_items(ic):
        pso = psB.tile([P, COLS], F32, tag="pt", name=f"pso{ic}")
        cpo = pc[0].tile([P, COLS], F32, tag=f"ocp{ic}", name=f"ocp{ic}")
        cp_out[ic] = cpo
        psc = psB.tile([P, IL], F32, tag="pv", name=f"psc{ic}")
        cpc = pc[0].tile([P, COLS], F32, tag=f"ccp{ic}", name=f"ccp{ic}")
        cp_ctx[ic] = cpc

        def mko(k):
            def it():
                nc.tensor.matmul(pso[:], u_sb[k][:, ic * P:(ic + 1) * P],
                                 wout_sb[k][:], start=(k == 0), stop=(k == 5))
                if k == 5:
                    nc.vector.tensor_add(cpo[:], pso[:], bout_bc[:])
            return it

        def mkc(k):
            def it():
                nc.tensor.matmul(psc[:], w_sb[k][:, ic * P:(ic + 1) * P],
                                 cwout_sb[k][:], start=(k == 0), stop=(k == 5))
                if k == 5:
                    nc.vector.tensor_add(cpc[:], psc[:], cbout_bc[:])
            return it
        return ([mko(k) for k in range(6)], [mkc(k) for k in range(6)])

    def chain(tail, post):
        def f():
            tail()
            post()
        return f

    def post_u3():
        nc.gpsimd.collective_compute(
            "AllGather", mybir.AluOpType.bypass,
            replica_groups=GROUPS,
            ins=[T["uwl"][3][0:128, :]],
            outs=[T["uwa3u"][:]],
        )
        for j in range(2):
            t = pu.tile([P, N], BF16, tag=f"usb{6 + j}", name=f"usb{6 + j}")
            nc.sync.dma_start(t[:], T["uwa3u"][j * P:(j + 1) * P, :])
            u_sb[6 + j] = t

    def post_wh0():
        nc.gpsimd.collective_compute(
            "AllGather", mybir.AluOpType.bypass,
            replica_groups=GROUPS,
            ins=[T["uwl"][3][128:128 + DH, :]],
            outs=[T["uwa3wh"][0][:]],
        )

    def post_wh1():
        nc.gpsimd.collective_compute(
            "AllGather", mybir.AluOpType.bypass,
            replica_groups=GROUPS,
            ins=[T["uwl"][3][128 + DH:256, :]],
            outs=[T["uwa3wh"][1][:]],
        )
        w6 = pu.tile([P, N], BF16, tag="wsb6")
        w7 = pu.tile([P, N], BF16, tag="wsb7")
        for hh in range(2):
            nc.sync.dma_start(w6[hh * DH:(hh + 1) * DH, :],
                              T["uwa3wh"][hh][0:DH, :])
            nc.sync.dma_start(w7[hh * DH:(hh + 1) * DH, :],
                              T["uwa3wh"][hh][DH:P, :])
        w_sb[6], w_sb[7] = w6, w7

    def make_post_pair(p):
        def f():
            nc.gpsimd.collective_compute(
                "AllGather", mybir.AluOpType.bypass,
                replica_groups=GROUPS,
                ins=[T["uwl"][p][:]],
                outs=[T["uwa"][p][:]],
            )
            load_pair(p)
            if p == 2:
                # projections are done by now; free their input pools and
                # queue the partial output projections as pair-3 fillers.
                drain_fillers()
                iostack.close()
                pc[0] = stack.enter_context(tc.tile_pool(name="pc", bufs=1))
                for ic in range(ICH):
                    oi, ci = partial_items(ic)
                    res[f"op{ic}"] = oi
                    res[f"cp{ic}"] = ci
                    drip.append(f"op{ic}")
                    drip.append(f"cp{ic}")
        return f

    pending = None
    for p in range(PAIRS):
        nf = 2 if p == 0 else (3 if p < 3 else 4)
        pending = block(p, 0, 0, pending, nf)
        pending = block(p, 0, 1, pending, nf)
        if p == 3:
            pending = chain(pending, post_u3)
        pending = block(p, 1, 0, pending, nf)
        if p == 3:
            pending = chain(pending, post_wh0)
        pending = block(p, 1, 1, pending, nf)
        pending = chain(pending, make_post_pair(p) if p < 3 else post_wh1)
    pending()

    drain_fillers()
    bstack.close()   # free psB PSUM for the finals

    # ---------------- phase C: gather-dependent finals only ----------------
    with tc.tile_pool(name="po", bufs=6) as po, \
         tc.tile_pool(name="psD", bufs=4, space="PSUM") as psD:
        for ic in range(ICH):
            ps = psD.tile([P, COLS], F32, tag="od", name=f"outf{ic}")
            for k in (6, 7):
                nc.tensor.matmul(ps[:], u_sb[k][:, ic * P:(ic + 1) * P],
                                 wout_sb[k][:], start=(k == 6), stop=(k == 7))
            o = po.tile([P, COLS], F32, tag="ot")
            nc.vector.tensor_add(o[:], ps[:], cp_out[ic][:])
            (nc.sync if ic % 2 else nc.scalar).dma_start(
                T["out_cols"][ic * P:(ic + 1) * P, :], o[:])
        for ic in range(ICH):
            ps = psD.tile([P, COLS], F32, tag="od", name=f"ctxf{ic}")
            for j, (k, r0) in enumerate(
                    ((6, 0), (7, 0), (6, DH), (7, DH))):
                nc.tensor.matmul(ps[:], w_sb[k][r0:r0 + DH, ic * P:(ic + 1) * P],
                                 cwout_sb[k][r0:r0 + DH, :],
                                 start=(j == 0), stop=(j == 3))
            o = po.tile([P, COLS], F32, tag="ot")
            nc.vector.tensor_add(o[:], ps[:], cp_ctx[ic][:])
            (nc.sync if ic % 2 else nc.scalar).dma_start(
                T["ctx_cols"][ic * P:(ic + 1) * P, :], o[:])
    stack.close()


def _get_nc():
    global _CACHED_NC
    if _CACHED_NC is None:
        _CACHED_NC = _build_nc()
    return _CACHED_NC


def _reorder_rows(w):
    """Reorder [INNER, :] rows to the uw_all K-chunk order (p-major, group X)."""
    chunks = []
    for p in range(4):
        for X in range(2):
            chunks.append(w[X * 512 + p * 128:X * 512 + (p + 1) * 128])
    return np.concatenate(chunks, axis=0)


def kernel(x, context, w_qk, w_v, cw_qk, cw_v, w_out, b_out, cw_out, cb_out):
    x = np.asarray(x, dtype=np.float32)
    context = np.asarray(context, dtype=np.float32)
    w_qk = np.asarray(w_qk, dtype=np.float32)
    w_v = np.asarray(w_v, dtype=np.float32)
    cw_qk = np.asarray(cw_qk, dtype=np.float32)
    cw_v = np.asarray(cw_v, dtype=np.float32)
    w_out_r = _reorder_rows(np.asarray(w_out, dtype=np.float32)).astype(ml_dtypes.bfloat16)
    cw_out_r = _reorder_rows(np.asarray(cw_out, dtype=np.float32)).astype(ml_dtypes.bfloat16)
    b_out = np.asarray(b_out, dtype=np.float32)
    cb_out = np.asarray(cb_out, dtype=np.float32)

    in_maps = []
    for c in range(8):
        b, g = c // 2, c % 2
        sl = slice(g * IL, (g + 1) * IL)
        in_maps.append({
            "xT": np.ascontiguousarray(x[b].T).astype(ml_dtypes.bfloat16),
            "ctxT": np.ascontiguousarray(context[b].T).astype(ml_dtypes.bfloat16),
            "wqk": np.ascontiguousarray(w_qk[:, sl]).astype(ml_dtypes.bfloat16),
            "wv": np.ascontiguousarray(w_v[:, sl]).astype(ml_dtypes.bfloat16),
            "cwqk": np.ascontiguousarray(cw_qk[:, sl]).astype(ml_dtypes.bfloat16),
            "cwv": np.ascontiguousarray(cw_v[:, sl]).astype(ml_dtypes.bfloat16),
            "wout": np.ascontiguousarray(w_out_r[:, sl]),
            "cwout": np.ascontiguousarray(cw_out_r[:, sl]),
            "bout": np.ascontiguousarray(b_out[None, sl]),
            "cbout": np.ascontiguousarray(cb_out[None, sl]),
        })

    nc = _get_nc()
    res = run_bass_kernel_spmd(nc, in_maps, list(range(8)))

    out = np.empty((B, N, DIM), dtype=np.float32)
    ctx_out = np.empty((B, N, DIM), dtype=np.float32)
    for b in range(B):
        out[b, :, 0:COLS] = res.results[2 * b]["out_cols"]
        out[b, :, COLS:] = res.results[2 * b + 1]["out_cols"]
        ctx_out[b, :, 0:COLS] = res.results[2 * b]["ctx_cols"]
        ctx_out[b, :, COLS:] = res.results[2 * b + 1]["ctx_cols"]
    return out, ctx_out


# revision 26
# speedup vs baseline: 1.3256x; 1.0154x over previous
"""Bidirectional cross-attention kernel for 8 Trainium2 NeuronCores.

Sharding: core c = 2*b + g handles batch b with head-group g (8 of 16 heads).
Each core projects Q/K/V/CV for its 8 heads, computes both softmax
orientations of the shared similarity matrix, and forms the per-head
attention outputs U = attn @ cv and W = context_attn^T @ v (stored
transposed, pre-scaled by the softmax normalizers).  The two cores of a
batch exchange their U/W halves with pairwise AllGathers, after which each
core computes a disjoint 512-column slice of both final projections.

Schedule notes (v2):
 - Phase B (sim+exp+ladder) saturates the scalar engine (128 exps) while
   the PE has spare cycles; the remaining projection matmuls (QT/KT m1-3,
   V, CV) are drip-fed into the PE queue as FILLER between sim/ladder
   steps so both engines run continuously from ~8us onward, instead of a
   serial 74us projection phase with the scalar engine idle.
 - (pair, orientation, head) blocks run sequentially so only one [65,N]
   U/W accumulator is live; PSUM = 2 sim bufs + accumulator + 2 small
   projection psums = exactly 8 banks.
 - Norm path: gpsimd copies the ones-row out of PSUM, DMA roundtrips the
   broadcast, vector does reciprocal + (psum x recip) -> bf16; no
   [65,1024] staging copies.
 - Pair 3's exchanges are split (U, W-head0, W-head1) so only 8 K=64
   matmuls + adds depend on the last gather.
"""

import os
import sys
from collections import deque

import numpy as np

for _p in ("/opt/trn_rl_repo", "/root/.axon_site/_ro/trn_rl_repo"):
    if os.path.isdir(_p) and _p not in sys.path:
        sys.path.append(_p)

import ml_dtypes  # noqa: E402
import concourse.bass as bass  # noqa: E402
import concourse.mybir as mybir  # noqa: E402
import concourse.tile as tile  # noqa: E402
from concourse import bacc  # noqa: E402
from concourse.bass_utils import run_bass_kernel_spmd  # noqa: E402

B, N, DIM = 4, 1024, 1024
H, DH = 16, 64
HL = 8            # heads per core
IL = HL * DH      # local inner width (512)
COLS = 512        # output columns per core
P = 128
PAIRS = HL // 2   # head pairs per core
KCH = DIM // P    # contraction chunks (8)
ICH = N // P      # sequence chunks (8)
SCALE = DH ** -0.5
GROUPS = [[0, 1], [2, 3], [4, 5], [6, 7]]
hs = 65           # head stride in V/CV tiles (64 values + ones column)

F32 = mybir.dt.float32
BF16 = mybir.dt.bfloat16
EXP = mybir.ActivationFunctionType.Exp

_CACHED_NC = None


def _build_nc():
    nc = bacc.Bacc("TRN2", target_bir_lowering=False, debug=False, num_devices=8)

    T = {}
    for nm, shape, dt in (
            ("xT", [DIM, N], BF16), ("ctxT", [DIM, N], BF16),
            ("wqk", [DIM, IL], BF16), ("wv", [DIM, IL], BF16),
            ("cwqk", [DIM, IL], BF16), ("cwv", [DIM, IL], BF16),
            ("wout", [DIM, COLS], BF16), ("cwout", [DIM, COLS], BF16),
            ("bout", [1, COLS], F32), ("cbout", [1, COLS], F32)):
        T[nm] = nc.dram_tensor(nm, shape, dt, kind="ExternalInput")
    T["out_cols"] = nc.dram_tensor("out_cols", [N, COLS], F32, kind="ExternalOutput")
    T["ctx_cols"] = nc.dram_tensor("ctx_cols", [N, COLS], F32, kind="ExternalOutput")

    with tile.TileContext(nc) as tc:
        with tc.tile_pool(name="dram", bufs=1, space="DRAM") as dpool:
            T["uwl"] = [dpool.tile([256, N], BF16, tag=f"uwl{p}", name=f"uwl{p}")
                        for p in range(4)]
            T["uwa"] = [dpool.tile([512, N], BF16, tag=f"uwa{p}", name=f"uwa{p}")
                        for p in range(3)]
            T["uwa3u"] = dpool.tile([256, N], BF16, tag="uwa3u", name="uwa3u")
            T["uwa3wh"] = [dpool.tile([128, N], BF16, tag=f"uwa3wh{h}",
                                      name=f"uwa3wh{h}")
                           for h in range(2)]
            T["normd"] = dpool.tile([16, N], F32, tag="normd", name="normd")
            _build_body(nc, tc, T)
    nc.compile()
    if os.environ.get("KERNEL_LDW_DEDUP", "1") == "1":
        _dedupe_ldweights(nc)
    return nc


def _dedupe_ldweights(nc):
    """Drop PE Ldweights that reload the exact weights already resident."""
    def sig(i):
        a = i.ins[0]
        return (a.memref, a.offset, str(a.ap), str(a.dtype),
                str(i.tile_position), str(i.tile_size),
                str(i.perf_mode), str(i.is_transpose))

    removed = 0
    for fn in nc.m.functions:
        for bb in fn.blocks:
            last = None
            keep = []
            for i in bb.instructions:
                if isinstance(i, mybir.InstLdweights):
                    s = sig(i)
                    si = i.sync_info
                    if s == last and (si is None or
                                      (not si.on_wait and not si.on_update)):
                        removed += 1
                        continue
                    last = s
                elif isinstance(i, mybir.InstMatmult):
                    pass
                elif getattr(i, "engine", None) == mybir.EngineType.PE:
                    last = None
                keep.append(i)
            if removed:
                bb.instructions = keep
    return removed


def _build_body(nc, tc, T):
    from contextlib import ExitStack
    stack = ExitStack()       # pools that live to the end
    bstack = ExitStack()      # PSUM pool released before the finals
    iostack = ExitStack()     # input pools released after the projections
    pqk = stack.enter_context(tc.tile_pool(name="pqk", bufs=1))
    pv = stack.enter_context(tc.tile_pool(name="pv", bufs=1))
    pf = stack.enter_context(tc.tile_pool(name="pf", bufs=1))
    pu = stack.enter_context(tc.tile_pool(name="pu", bufs=1))
    pe = stack.enter_context(tc.tile_pool(name="pe", bufs=4))
    pn = stack.enter_context(tc.tile_pool(name="pn", bufs=2))
    pn1 = stack.enter_context(tc.tile_pool(name="pn1", bufs=1))
    psB = bstack.enter_context(tc.tile_pool(name="psB", bufs=1, space="PSUM"))
    pw = iostack.enter_context(tc.tile_pool(name="pw", bufs=1))
    pin = iostack.enter_context(tc.tile_pool(name="pin", bufs=1))

    # ---------------- input DMA ----------------
    # critical path: wqk + xT (sync queue) and cwqk + ctxT (scalar queue,
    # which is idle until phase B starts).
    wqk_t, xt, cwqk_t, ct = [], [], [], []
    for k in range(KCH):
        w = pw.tile([P, IL], BF16, tag=f"wqk{k}", name=f"wqk{k}")
        nc.gpsimd.dma_start(w[:], T["wqk"][k * P:(k + 1) * P, :])
        wqk_t.append(w)
        t = pin.tile([P, N], BF16, tag=f"xT{k}", name=f"xt{k}")
        nc.sync.dma_start(t[:], T["xT"][k * P:(k + 1) * P, :])
        xt.append(t)
        w = pw.tile([P, IL], BF16, tag=f"cwqk{k}", name=f"cwqk{k}")
        nc.scalar.dma_start(w[:], T["cwqk"][k * P:(k + 1) * P, :])
        cwqk_t.append(w)
    for k in range(KCH):
        t = pin.tile([P, N], BF16, tag=f"cT{k}", name=f"ct{k}")
        (nc.gpsimd if k % 2 else nc.scalar).dma_start(
            t[:], T["ctxT"][k * P:(k + 1) * P, :])
        ct.append(t)
    wv_t, cwv_t = [], []
    for k in range(KCH):
        w = pw.tile([P, IL], BF16, tag=f"cwv{k}", name=f"cwv{k}")
        nc.gpsimd.dma_start(w[:], T["cwv"][k * P:(k + 1) * P, :])
        cwv_t.append(w)
    for k in range(KCH):
        w = pw.tile([P, IL], BF16, tag=f"wv{k}", name=f"wv{k}")
        nc.gpsimd.dma_start(w[:], T["wv"][k * P:(k + 1) * P, :])
        wv_t.append(w)
    # output-side weights/biases (needed in phase C only)
    bout_bc = pf.tile([P, COLS], F32, tag="bb")
    nc.gpsimd.dma_start(bout_bc[:], T["bout"][:].to_broadcast((P, COLS)))
    cbout_bc = pf.tile([P, COLS], F32, tag="cbb")
    nc.gpsimd.dma_start(cbout_bc[:], T["cbout"][:].to_broadcast((P, COLS)))
    wout_sb, cwout_sb = [], []
    for k in range(KCH):
        t = pf.tile([P, COLS], BF16, tag=f"wo{k}")
        nc.gpsimd.dma_start(t[:], T["wout"][k * P:(k + 1) * P, :])
        wout_sb.append(t)
        t = pf.tile([P, COLS], BF16, tag=f"cwo{k}")
        nc.gpsimd.dma_start(t[:], T["cwout"][k * P:(k + 1) * P, :])
        cwout_sb.append(t)

    # ---------------- projection emitters (filler items) ----------------
    QT = [None] * PAIRS   # QT[m] = (pa, pb): head A rows 0:64 / head B 64:128
    KT = [None] * PAIRS
    V = [None] * ICH      # [128, HL*hs] bf16, ones col per head
    CV = [None] * ICH

    def emit_projT(src, wtiles, store, tag):
        """Chunk-major full projection: all four head-pairs accumulate at
        once (PSUM slots sim0/sim1/uw hold m0-2, pt+pv hold m3's halves)
        so the PE consumes each arriving DMA chunk with 8 matmuls."""
        pab = []
        for m in range(PAIRS):
            pa = pqk.tile([P, N], BF16, tag=f"{tag}a{m}", name=f"{tag}a{m}")
            pb = pqk.tile([P, N], BF16, tag=f"{tag}b{m}", name=f"{tag}b{m}")
            store[m] = (pa, pb)
            pab.append((pa, pb))
            nc.vector.memset(pa[DH:P, :], 0.0)
            nc.vector.memset(pb[0:DH, :], 0.0)
        big = [psB.tile([P, N], F32, tag=t, name=f"{tag}_ps{t}")
               for t in ("sim0", "sim1", "uw")]
        m3 = [psB.tile([P, COLS], F32, tag=t, name=f"{tag}_ps{t}")
              for t in ("pt", "pv")]
        for k in range(KCH):
            st, sp = (k == 0), (k == KCH - 1)
            for m in range(3):
                lhsT = wtiles[k][:, m * P:(m + 1) * P]
                for half in range(2):
                    nc.tensor.matmul(
                        big[m][:, half * COLS:(half + 1) * COLS], lhsT,
                        src[k][:, half * COLS:(half + 1) * COLS],
                        start=st, stop=sp)
            lhsT = wtiles[k][:, 3 * P:4 * P]
            for half in range(2):
                nc.tensor.matmul(m3[half][:], lhsT,
                                 src[k][:, half * COLS:(half + 1) * COLS],
                                 start=st, stop=sp)
        for m in range(3):
            pa, pb = pab[m]
            nc.vector.tensor_copy(pa[0:DH, :], big[m][0:DH, :])
            nc.vector.tensor_copy(pb[DH:P, :], big[m][DH:P, :])
        pa, pb = pab[3]
        for half in range(2):
            lo = half * COLS
            nc.vector.tensor_copy(pa[0:DH, lo:lo + COLS], m3[half][0:DH, :])
            nc.vector.tensor_copy(pb[DH:P, lo:lo + COLS], m3[half][DH:P, :])

    def projV_items(src, wtiles, store, ic, tag):
        o = pv.tile([P, HL * hs], BF16, tag=f"{tag}{ic}")
        store[ic] = o
        ps = psB.tile([P, IL], F32, tag="pv", name=f"pv_{tag}{ic}")
        items = []

        def mk(k):
            def it():
                nc.tensor.matmul(ps[:], src[k][:, ic * P:(ic + 1) * P],
                                 wtiles[k][:],
                                 start=(k == 0), stop=(k == KCH - 1))
                if k == KCH - 1:
                    dst = o[:].rearrange("p (h e) -> p h e", e=hs)
                    nc.vector.tensor_copy(
                        dst[:, :, 0:DH],
                        ps[:].rearrange("p (h e) -> p h e", e=DH))
                    nc.vector.memset(dst[:, :, DH:hs], 1.0)
            return it
        items.extend(mk(k) for k in range(KCH))
        return items

    # resource name -> remaining items; drip order for background filling
    res = {}
    for ic in range(ICH):
        res[f"cv{ic}"] = projV_items(ct, cwv_t, CV, ic, "cv")
    for ic in range(ICH):
        res[f"v{ic}"] = projV_items(xt, wv_t, V, ic, "v")
    drip = deque([f"cv{ic}" for ic in range(ICH)] +
                 [f"v{ic}" for ic in range(ICH)])

    def require(name):
        for it in res.pop(name, ()):
            it()

    def emit_fillers(n):
        done = 0
        while done < n and drip:
            lst = res.get(drip[0])
            if not lst:
                res.pop(drip[0], None)
                drip.popleft()
                continue
            lst.pop(0)()
            done += 1

    def drain_fillers():
        while drip:
            emit_fillers(len(drip) * 32)

    # ---------------- prelude: full QT/KT, DMA-arrival paced ----------------
    emit_projT(xt, wqk_t, QT, "qt")
    emit_projT(ct, cwqk_t, KT, "kt")

    # ---------------- phase B ----------------
    u_sb = [None] * KCH
    w_sb = [None] * KCH

    def norm(p, orient, hh, acc):
        """acc: [65,N] psum; rows 0:64 data, row 64 normalizer."""
        slot = 4 * p + 2 * orient + hh
        rst = pn1.tile([DH + 1, N], F32, tag="rst")
        nc.vector.tensor_copy(rst[:], acc[0:DH + 1, :])
        nc.sync.dma_start(T["normd"][slot:slot + 1, :], rst[DH:DH + 1, :])
        rbc = pn.tile([DH, N], F32, tag="rbc")
        nc.sync.dma_start(
            rbc[:], T["normd"][slot:slot + 1, :].to_broadcast((DH, N)))
        nc.vector.reciprocal_approx_fast(rbc[:], rbc[:])
        ubf = pn.tile([DH, N], BF16, tag="ubf")
        nc.vector.tensor_mul(ubf[:], rst[0:DH, :], rbc[:])
        r0 = orient * 128 + hh * DH
        nc.sync.dma_start(T["uwl"][p][r0:r0 + DH, :], ubf[:])

    def block(p, orient, hh, pending, nfill=2):
        """One (pair, orientation, head) unit: 8 sim+exp steps with the
        U/W ladder at lag 2 and fillers padding the PE.  The tail
        (last two ladder steps + norm) is returned as a closure and runs
        inside the NEXT block's first step, so the exp(7) -> ladder(7)
        chain never delays the next block's first sim/exp."""
        h = 2 * p + hh
        if orient == 0:   # U: simT (j on partitions), ladder vs CV
            lhsT_full, rhs_full = KT[p][hh], QT[p][hh]
            lad = CV
        else:             # W: sim (i on partitions), ladder vs V
            lhsT_full, rhs_full = QT[p][hh], KT[p][hh]
            lad = V
        acc = psB.tile([P, N], F32, tag="uw", name=f"uw{p}_{orient}_{hh}")
        E = [None] * ICH

        def ladder(jc):
            require(f"{'cv' if orient == 0 else 'v'}{jc}")
            lt = lad[jc][:, h * hs:(h + 1) * hs]
            for half in range(2):
                nc.tensor.matmul(
                    acc[0:hs, half * COLS:(half + 1) * COLS],
                    lt, E[jc][:, half * COLS:(half + 1) * COLS],
                    start=(jc == 0), stop=(jc == ICH - 1))

        for jc in range(ICH):
            ps = psB.tile([P, N], F32, tag=f"sim{jc & 1}")
            lhsT = lhsT_full[:, jc * P:(jc + 1) * P]
            for half in range(2):
                nc.tensor.matmul(ps[:, half * COLS:(half + 1) * COLS],
                                 lhsT, rhs_full[:, half * COLS:(half + 1) * COLS],
                                 start=True, stop=True)
            e = pe.tile([P, N], BF16, tag="E")
            nc.scalar.activation(e[:], ps[:], EXP, scale=SCALE)
            E[jc] = e
            if jc == 0 and pending is not None:
                pending()
            if jc >= 2:
                ladder(jc - 2)
            emit_fillers(nfill)

        def tail():
            ladder(ICH - 2)
            ladder(ICH - 1)
            norm(p, orient, hh, acc)
        return tail

    def load_pair(p):
        rows = ((0, u_sb, "u", 2 * p), (256, u_sb, "u", 2 * p + 1),
                (128, w_sb, "w", 2 * p), (384, w_sb, "w", 2 * p + 1))
        for row, arr, nm, k in rows:
            t = pu.tile([P, N], BF16, tag=f"{nm}sb{k}", name=f"{nm}sb{k}")
            nc.sync.dma_start(t[:], T["uwa"][p][row:row + P, :])
            arr[k] = t

    # partial output projections (k chunks 0..5, from pairs 0-2) run as
    # pair-3 fillers in the pt/pv psum slots freed by the projections.
    cp_out = [None] * ICH
    cp_ctx = [None] * ICH
    pc = [None]

    def partial_items(ic):
        pso = psB.tile([P, COLS], F32, tag="pt", name=f"pso{ic}")
        cpo = pc[0].tile([P, COLS], F32, tag=f"ocp{ic}", name=f"ocp{ic}")
        cp_out[ic] = cpo

        def mko(k):
            def it():
                nc.tensor.matmul(pso[:], u_sb[k][:, ic * P:(ic + 1) * P],
                                 wout_sb[k][:], start=(k == 0), stop=(k == 5))
                if k == 5:
                    nc.vector.tensor_add(cpo[:], pso[:], bout_bc[:])
            return it
        return [mko(k) for k in range(6)]

    def chain(tail, post):
        def f():
            tail()
            post()
        return f

    def post_u3():
        nc.gpsimd.collective_compute(
            "AllGather", mybir.AluOpType.bypass,
            replica_groups=GROUPS,
            ins=[T["uwl"][3][0:128, :]],
            outs=[T["uwa3u"][:]],
        )
        for j in range(2):
            t = pu.tile([P, N], BF16, tag=f"usb{6 + j}", name=f"usb{6 + j}")
            nc.sync.dma_start(t[:], T["uwa3u"][j * P:(j + 1) * P, :])
            u_sb[6 + j] = t

    def post_wh0():
        nc.gpsimd.collective_compute(
            "AllGather", mybir.AluOpType.bypass,
            replica_groups=GROUPS,
            ins=[T["uwl"][3][128:128 + DH, :]],
            outs=[T["uwa3wh"][0][:]],
        )

    def post_wh1():
        nc.gpsimd.collective_compute(
            "AllGather", mybir.AluOpType.bypass,
            replica_groups=GROUPS,
            ins=[T["uwl"][3][128 + DH:256, :]],
            outs=[T["uwa3wh"][1][:]],
        )
        w6 = pu.tile([P, N], BF16, tag="wsb6")
        w7 = pu.tile([P, N], BF16, tag="wsb7")
        for hh in range(2):
            nc.sync.dma_start(w6[hh * DH:(hh + 1) * DH, :],
                              T["uwa3wh"][hh][0:DH, :])
            nc.sync.dma_start(w7[hh * DH:(hh + 1) * DH, :],
                              T["uwa3wh"][hh][DH:P, :])
        w_sb[6], w_sb[7] = w6, w7

    def make_post_pair(p):
        def f():
            nc.gpsimd.collective_compute(
                "AllGather", mybir.AluOpType.bypass,
                replica_groups=GROUPS,
                ins=[T["uwl"][p][:]],
                outs=[T["uwa"][p][:]],
            )
            load_pair(p)
            if p == 2:
                # projections are done by now; free their input pools and
                # queue the partial output projections as pair-3 fillers.
                drain_fillers()
                iostack.close()
                pc[0] = stack.enter_context(tc.tile_pool(name="pc", bufs=1))
                for ic in range(ICH):
                    res[f"op{ic}"] = partial_items(ic)
                    drip.append(f"op{ic}")
        return f

    pending = None
    for p in range(PAIRS):
        nf = 2 if p == 0 else (3 if p < 3 else 4)
        pending = block(p, 0, 0, pending, nf)
        pending = block(p, 0, 1, pending, nf)
        if p == 3:
            pending = chain(pending, post_u3)
        pending = block(p, 1, 0, pending, nf)
        if p == 3:
            pending = chain(pending, post_wh0)
        pending = block(p, 1, 1, pending, nf)
        pending = chain(pending, make_post_pair(p) if p < 3 else post_wh1)
    pending()

    drain_fillers()
    bstack.close()   # free psB PSUM for the finals

    # ---------------- finals: ctx partials shadow the last gather ----------
    with tc.tile_pool(name="psD", bufs=4, space="PSUM") as psD:
        for ic in range(ICH):
            psc = psD.tile([P, COLS], F32, tag="od", name=f"cpp{ic}")
            for k in range(6):
                nc.tensor.matmul(psc[:], w_sb[k][:, ic * P:(ic + 1) * P],
                                 cwout_sb[k][:], start=(k == 0), stop=(k == 5))
            cpc = pc[0].tile([P, COLS], F32, tag=f"ccp{ic}", name=f"ccp{ic}")
            nc.vector.tensor_add(cpc[:], psc[:], cbout_bc[:])
            cp_ctx[ic] = cpc
            ps = psD.tile([P, COLS], F32, tag="od", name=f"outf{ic}")
            for k in (6, 7):
                nc.tensor.matmul(ps[:], u_sb[k][:, ic * P:(ic + 1) * P],
                                 wout_sb[k][:], start=(k == 6), stop=(k == 7))
            o = pc[0].tile([P, COLS], F32, tag=f"ot{ic % 4}", name=f"oo{ic}")
            nc.vector.tensor_add(o[:], ps[:], cp_out[ic][:])
            (nc.sync if ic % 2 else nc.scalar).dma_start(
                T["out_cols"][ic * P:(ic + 1) * P, :], o[:])
        for ic in range(ICH):
            ps = psD.tile([P, COLS], F32, tag="od", name=f"ctxf{ic}")
            for j, (k, r0) in enumerate(((6, 0), (7, 0), (6, DH), (7, DH))):
                nc.tensor.matmul(ps[:], w_sb[k][r0:r0 + DH, ic * P:(ic + 1) * P],
                                 cwout_sb[k][r0:r0 + DH, :],
                                 start=(j == 0), stop=(j == 3))
            o = pc[0].tile([P, COLS], F32, tag=f"ot{ic % 4}", name=f"co{ic}")
            nc.vector.tensor_add(o[:], ps[:], cp_ctx[ic][:])
            (nc.sync if ic % 2 else nc.scalar).dma_start(
                T["ctx_cols"][ic * P:(ic + 1) * P, :], o[:])
    stack.close()


def _get_nc():
    global _CACHED_NC
    if _CACHED_NC is None:
        _CACHED_NC = _build_nc()
    return _CACHED_NC


def _reorder_rows(w):
    """Reorder [INNER, :] rows to the uw_all K-chunk order (p-major, group X)."""
    chunks = []
    for p in range(4):
        for X in range(2):
            chunks.append(w[X * 512 + p * 128:X * 512 + (p + 1) * 128])
    return np.concatenate(chunks, axis=0)


def kernel(x, context, w_qk, w_v, cw_qk, cw_v, w_out, b_out, cw_out, cb_out):
    x = np.asarray(x, dtype=np.float32)
    context = np.asarray(context, dtype=np.float32)
    w_qk = np.asarray(w_qk, dtype=np.float32)
    w_v = np.asarray(w_v, dtype=np.float32)
    cw_qk = np.asarray(cw_qk, dtype=np.float32)
    cw_v = np.asarray(cw_v, dtype=np.float32)
    w_out_r = _reorder_rows(np.asarray(w_out, dtype=np.float32)).astype(ml_dtypes.bfloat16)
    cw_out_r = _reorder_rows(np.asarray(cw_out, dtype=np.float32)).astype(ml_dtypes.bfloat16)
    b_out = np.asarray(b_out, dtype=np.float32)
    cb_out = np.asarray(cb_out, dtype=np.float32)

    in_maps = []
    for c in range(8):
        b, g = c // 2, c % 2
        sl = slice(g * IL, (g + 1) * IL)
        in_maps.append({
            "xT": np.ascontiguousarray(x[b].T).astype(ml_dtypes.bfloat16),
            "ctxT": np.ascontiguousarray(context[b].T).astype(ml_dtypes.bfloat16),
            "wqk": np.ascontiguousarray(w_qk[:, sl]).astype(ml_dtypes.bfloat16),
            "wv": np.ascontiguousarray(w_v[:, sl]).astype(ml_dtypes.bfloat16),
            "cwqk": np.ascontiguousarray(cw_qk[:, sl]).astype(ml_dtypes.bfloat16),
            "cwv": np.ascontiguousarray(cw_v[:, sl]).astype(ml_dtypes.bfloat16),
            "wout": np.ascontiguousarray(w_out_r[:, sl]),
            "cwout": np.ascontiguousarray(cw_out_r[:, sl]),
            "bout": np.ascontiguousarray(b_out[None, sl]),
            "cbout": np.ascontiguousarray(cb_out[None, sl]),
        })

    nc = _get_nc()
    res = run_bass_kernel_spmd(nc, in_maps, list(range(8)))

    out = np.empty((B, N, DIM), dtype=np.float32)
    ctx_out = np.empty((B, N, DIM), dtype=np.float32)
    for b in range(B):
        out[b, :, 0:COLS] = res.results[2 * b]["out_cols"]
        out[b, :, COLS:] = res.results[2 * b + 1]["out_cols"]
        ctx_out[b, :, 0:COLS] = res.results[2 * b]["ctx_cols"]
        ctx_out[b, :, COLS:] = res.results[2 * b + 1]["ctx_cols"]
    return out, ctx_out


# revision 28
# speedup vs baseline: 1.4543x; 1.0971x over previous
"""Bidirectional cross-attention kernel for 8 Trainium2 NeuronCores.

Sharding: core c = 2*b + g handles batch b with head-group g (8 of 16 heads).
Each core projects Q/K/V/CV for its 8 heads, computes both softmax
orientations of the shared similarity matrix, and forms the per-head
attention outputs U = attn @ cv and W = context_attn^T @ v (stored
transposed, pre-scaled by the softmax normalizers).  The two cores of a
batch exchange their U/W halves with pairwise AllGathers, after which each
core computes a disjoint 512-column slice of both final projections.

Schedule notes (v2):
 - Phase B (sim+exp+ladder) saturates the scalar engine (128 exps) while
   the PE has spare cycles; the remaining projection matmuls (QT/KT m1-3,
   V, CV) are drip-fed into the PE queue as FILLER between sim/ladder
   steps so both engines run continuously from ~8us onward, instead of a
   serial 74us projection phase with the scalar engine idle.
 - (pair, orientation, head) blocks run sequentially so only one [65,N]
   U/W accumulator is live; PSUM = 2 sim bufs + accumulator + 2 small
   projection psums = exactly 8 banks.
 - Norm path: gpsimd copies the ones-row out of PSUM, DMA roundtrips the
   broadcast, vector does reciprocal + (psum x recip) -> bf16; no
   [65,1024] staging copies.
 - Pair 3's exchanges are split (U, W-head0, W-head1) so only 8 K=64
   matmuls + adds depend on the last gather.
"""

import os
import sys
from collections import deque

import numpy as np

for _p in ("/opt/trn_rl_repo", "/root/.axon_site/_ro/trn_rl_repo"):
    if os.path.isdir(_p) and _p not in sys.path:
        sys.path.append(_p)

import ml_dtypes  # noqa: E402
import concourse.bass as bass  # noqa: E402
import concourse.mybir as mybir  # noqa: E402
import concourse.tile as tile  # noqa: E402
from concourse import bacc  # noqa: E402
from concourse.bass_utils import run_bass_kernel_spmd  # noqa: E402

B, N, DIM = 4, 1024, 1024
H, DH = 16, 64
HL = 8            # heads per core
IL = HL * DH      # local inner width (512)
COLS = 512        # output columns per core
P = 128
PAIRS = HL // 2   # head pairs per core
KCH = DIM // P    # contraction chunks (8)
ICH = N // P      # sequence chunks (8)
SCALE = DH ** -0.5
GROUPS = [[0, 1], [2, 3], [4, 5], [6, 7]]
hs = 65           # head stride in V/CV tiles (64 values + ones column)

F32 = mybir.dt.float32
BF16 = mybir.dt.bfloat16
EXP = mybir.ActivationFunctionType.Exp

_CACHED_NC = None


def _build_nc():
    nc = bacc.Bacc("TRN2", target_bir_lowering=False, debug=False, num_devices=8)

    T = {}
    for nm, shape, dt in (
            ("xT", [DIM, N], BF16), ("ctxT", [DIM, N], BF16),
            ("wqk", [DIM, IL], BF16), ("wv", [DIM, IL], BF16),
            ("cwqk", [DIM, IL], BF16), ("cwv", [DIM, IL], BF16),
            ("wout", [DIM, COLS], BF16), ("cwout", [DIM, COLS], BF16),
            ("bout", [1, COLS], F32), ("cbout", [1, COLS], F32)):
        T[nm] = nc.dram_tensor(nm, shape, dt, kind="ExternalInput")
    T["out_cols"] = nc.dram_tensor("out_cols", [N, COLS], F32, kind="ExternalOutput")
    T["ctx_cols"] = nc.dram_tensor("ctx_cols", [N, COLS], F32, kind="ExternalOutput")

    with tile.TileContext(nc) as tc:
        with tc.tile_pool(name="dram", bufs=1, space="DRAM") as dpool:
            T["uwl"] = [dpool.tile([256, N], BF16, tag=f"uwl{p}", name=f"uwl{p}")
                        for p in range(4)]
            T["uwa"] = [dpool.tile([512, N], BF16, tag=f"uwa{p}", name=f"uwa{p}")
                        for p in range(3)]
            T["uwa3u"] = dpool.tile([256, N], BF16, tag="uwa3u", name="uwa3u")
            T["uwa3w"] = dpool.tile([256, N], BF16, tag="uwa3w", name="uwa3w")
            T["ccwarm"] = dpool.tile([2, 64], BF16, tag="ccwarm", name="ccwarm")
            T["normd"] = dpool.tile([16, N], F32, tag="normd", name="normd")
            _build_body(nc, tc, T)
    nc.compile()
    if os.environ.get("KERNEL_LDW_DEDUP", "1") == "1":
        _dedupe_ldweights(nc)
    return nc


def _dedupe_ldweights(nc):
    """Drop PE Ldweights that reload the exact weights already resident."""
    def sig(i):
        a = i.ins[0]
        return (a.memref, a.offset, str(a.ap), str(a.dtype),
                str(i.tile_position), str(i.tile_size),
                str(i.perf_mode), str(i.is_transpose))

    removed = 0
    for fn in nc.m.functions:
        for bb in fn.blocks:
            last = None
            keep = []
            for i in bb.instructions:
                if isinstance(i, mybir.InstLdweights):
                    s = sig(i)
                    si = i.sync_info
                    if s == last and (si is None or
                                      (not si.on_wait and not si.on_update)):
                        removed += 1
                        continue
                    last = s
                elif isinstance(i, mybir.InstMatmult):
                    pass
                elif getattr(i, "engine", None) == mybir.EngineType.PE:
                    last = None
                keep.append(i)
            if removed:
                bb.instructions = keep
    return removed


def _build_body(nc, tc, T):
    from contextlib import ExitStack
    stack = ExitStack()       # pools that live to the end
    bstack = ExitStack()      # PSUM pool released before the finals
    iostack = ExitStack()     # input pools released after the projections
    pqk = stack.enter_context(tc.tile_pool(name="pqk", bufs=1))
    pv = stack.enter_context(tc.tile_pool(name="pv", bufs=1))
    pf = stack.enter_context(tc.tile_pool(name="pf", bufs=1))
    pu = stack.enter_context(tc.tile_pool(name="pu", bufs=1))
    pe = stack.enter_context(tc.tile_pool(name="pe", bufs=4))
    pn = stack.enter_context(tc.tile_pool(name="pn", bufs=2))
    pn1 = stack.enter_context(tc.tile_pool(name="pn1", bufs=1))
    psB = bstack.enter_context(tc.tile_pool(name="psB", bufs=1, space="PSUM"))
    pw = iostack.enter_context(tc.tile_pool(name="pw", bufs=1))
    pin = iostack.enter_context(tc.tile_pool(name="pin", bufs=1))

    # warm up the collective engine (first cc pays ~11us init) while the
    # prelude runs; gathers its own scratch row.
    nc.gpsimd.collective_compute(
        "AllGather", mybir.AluOpType.bypass,
        replica_groups=GROUPS,
        ins=[T["ccwarm"][0:1, :]],
        outs=[T["ccwarm"][:]],
    )

    # ---------------- input DMA ----------------
    # critical path: wqk + xT (sync queue) and cwqk + ctxT (scalar queue,
    # which is idle until phase B starts).
    wqk_t, xt, cwqk_t, ct = [], [], [], []
    for k in range(KCH):
        w = pw.tile([P, IL], BF16, tag=f"wqk{k}", name=f"wqk{k}")
        nc.gpsimd.dma_start(w[:], T["wqk"][k * P:(k + 1) * P, :])
        wqk_t.append(w)
        t = pin.tile([P, N], BF16, tag=f"xT{k}", name=f"xt{k}")
        nc.sync.dma_start(t[:], T["xT"][k * P:(k + 1) * P, :])
        xt.append(t)
        w = pw.tile([P, IL], BF16, tag=f"cwqk{k}", name=f"cwqk{k}")
        nc.scalar.dma_start(w[:], T["cwqk"][k * P:(k + 1) * P, :])
        cwqk_t.append(w)
    for k in range(KCH):
        t = pin.tile([P, N], BF16, tag=f"cT{k}", name=f"ct{k}")
        (nc.gpsimd if k % 2 else nc.scalar).dma_start(
            t[:], T["ctxT"][k * P:(k + 1) * P, :])
        ct.append(t)
    wv_t, cwv_t = [], []
    for k in range(KCH):
        w = pw.tile([P, IL], BF16, tag=f"cwv{k}", name=f"cwv{k}")
        nc.gpsimd.dma_start(w[:], T["cwv"][k * P:(k + 1) * P, :])
        cwv_t.append(w)
    for k in range(KCH):
        w = pw.tile([P, IL], BF16, tag=f"wv{k}", name=f"wv{k}")
        nc.gpsimd.dma_start(w[:], T["wv"][k * P:(k + 1) * P, :])
        wv_t.append(w)
    # output-side weights/biases (needed in phase C only)
    bout_bc = pf.tile([P, COLS], F32, tag="bb")
    nc.gpsimd.dma_start(bout_bc[:], T["bout"][:].to_broadcast((P, COLS)))
    cbout_bc = pf.tile([P, COLS], F32, tag="cbb")
    nc.gpsimd.dma_start(cbout_bc[:], T["cbout"][:].to_broadcast((P, COLS)))
    wout_sb, cwout_sb = [], []
    for k in range(KCH):
        t = pf.tile([P, COLS], BF16, tag=f"wo{k}")
        nc.gpsimd.dma_start(t[:], T["wout"][k * P:(k + 1) * P, :])
        wout_sb.append(t)
        t = pf.tile([P, COLS], BF16, tag=f"cwo{k}")
        nc.gpsimd.dma_start(t[:], T["cwout"][k * P:(k + 1) * P, :])
        cwout_sb.append(t)

    # ---------------- projection emitters (filler items) ----------------
    QT = [None] * PAIRS   # QT[m] = (pa, pb): head A rows 0:64 / head B 64:128
    KT = [None] * PAIRS
    V = [None] * ICH      # [128, HL*hs] bf16, ones col per head
    CV = [None] * ICH

    def emit_projT(src, wtiles, store, tag):
        """Chunk-major full projection: all four head-pairs accumulate at
        once (PSUM slots sim0/sim1/uw hold m0-2, pt+pv hold m3's halves)
        so the PE consumes each arriving DMA chunk with 8 matmuls."""
        pab = []
        for m in range(PAIRS):
            pa = pqk.tile([P, N], BF16, tag=f"{tag}a{m}", name=f"{tag}a{m}")
            pb = pqk.tile([P, N], BF16, tag=f"{tag}b{m}", name=f"{tag}b{m}")
            store[m] = (pa, pb)
            pab.append((pa, pb))
            nc.vector.memset(pa[DH:P, :], 0.0)
            nc.vector.memset(pb[0:DH, :], 0.0)
        big = [psB.tile([P, N], F32, tag=t, name=f"{tag}_ps{t}")
               for t in ("sim0", "sim1", "uw")]
        for k in range(KCH):
            st, sp = (k == 0), (k == KCH - 1)
            for m in range(3):
                lhsT = wtiles[k][:, m * P:(m + 1) * P]
                for half in range(2):
                    nc.tensor.matmul(
                        big[m][:, half * COLS:(half + 1) * COLS], lhsT,
                        src[k][:, half * COLS:(half + 1) * COLS],
                        start=st, stop=sp)
        for m in range(3):
            pa, pb = pab[m]
            nc.vector.tensor_copy(pa[0:DH, :], big[m][0:DH, :])
            nc.vector.tensor_copy(pb[DH:P, :], big[m][DH:P, :])
        pa, pb = pab[3]
        for half in range(2):
            lo = half * COLS
            m3 = psB.tile([P, COLS], F32, tag="pt", name=f"{tag}_ps3{half}")
            for k in range(KCH):
                nc.tensor.matmul(m3[:], wtiles[k][:, 3 * P:4 * P],
                                 src[k][:, lo:lo + COLS],
                                 start=(k == 0), stop=(k == KCH - 1))
            nc.vector.tensor_copy(pa[0:DH, lo:lo + COLS], m3[0:DH, :])
            nc.vector.tensor_copy(pb[DH:P, lo:lo + COLS], m3[DH:P, :])

    def projV_items(src, wtiles, store, ic, pr, tag):
        """One pair's 2-head slice of a V/CV chunk: 8 narrow matmuls."""
        if store[ic] is None:
            store[ic] = pv.tile([P, HL * hs], BF16, tag=f"{tag}{ic}",
                                name=f"{tag}{ic}")
        o = store[ic]
        ps = psB.tile([P, 2 * DH], F32, tag="pv", name=f"pv_{tag}{ic}_{pr}")

        def mk(k):
            def it():
                nc.tensor.matmul(ps[:], src[k][:, ic * P:(ic + 1) * P],
                                 wtiles[k][:, 2 * pr * DH:2 * (pr + 1) * DH],
                                 start=(k == 0), stop=(k == KCH - 1))
                if k == KCH - 1:
                    dst = o[:].rearrange("p (h e) -> p h e", e=hs)
                    nc.vector.tensor_copy(
                        dst[:, 2 * pr:2 * pr + 2, 0:DH],
                        ps[:].rearrange("p (h e) -> p h e", e=DH))
                    nc.vector.memset(dst[:, 2 * pr:2 * pr + 2, DH:hs], 1.0)
            return it
        return [mk(k) for k in range(KCH)]

    def projT_m_items(src, wtiles, store, m, tag):
        """KT m1-3 built as fillers: two 8-matmul half passes via pt."""
        pa = pqk.tile([P, N], BF16, tag=f"{tag}a{m}", name=f"{tag}a{m}")
        pb = pqk.tile([P, N], BF16, tag=f"{tag}b{m}", name=f"{tag}b{m}")
        store[m] = (pa, pb)
        items = [lambda: (nc.vector.memset(pa[DH:P, :], 0.0),
                          nc.vector.memset(pb[0:DH, :], 0.0))]
        for half in range(2):
            ps = psB.tile([P, COLS], F32, tag="pt", name=f"pt_{tag}{m}_{half}")
            lo = half * COLS

            def mk(k, ps=ps, lo=lo):
                def it():
                    nc.tensor.matmul(ps[:], wtiles[k][:, m * P:(m + 1) * P],
                                     src[k][:, lo:lo + COLS],
                                     start=(k == 0), stop=(k == KCH - 1))
                    if k == KCH - 1:
                        nc.vector.tensor_copy(pa[0:DH, lo:lo + COLS],
                                              ps[0:DH, :])
                        nc.vector.tensor_copy(pb[DH:P, lo:lo + COLS],
                                              ps[DH:P, :])
                return it
            items.extend(mk(k) for k in range(KCH))
        return items

    # resource name -> remaining items; drip order for background filling
    res = {}
    for pr in range(PAIRS):
        for ic in range(ICH):
            res[f"cv{pr}_{ic}"] = projV_items(ct, cwv_t, CV, ic, pr, "cv")
            res[f"v{pr}_{ic}"] = projV_items(xt, wv_t, V, ic, pr, "v")
    drip = deque()

    def require(name):
        for it in res.pop(name, ()):
            it()

    def emit_fillers(n):
        done = 0
        while done < n and drip:
            lst = res.get(drip[0])
            if not lst:
                res.pop(drip[0], None)
                drip.popleft()
                continue
            lst.pop(0)()
            done += 1

    def drain_fillers():
        while drip:
            emit_fillers(len(drip) * 32)

    # ---------------- prelude: full QT + KT-m0, DMA-arrival paced ----------
    emit_projT(xt, wqk_t, QT, "qt")
    kpa = pqk.tile([P, N], BF16, tag="kta0", name="kta0")
    kpb = pqk.tile([P, N], BF16, tag="ktb0", name="ktb0")
    KT[0] = (kpa, kpb)
    nc.vector.memset(kpa[DH:P, :], 0.0)
    nc.vector.memset(kpb[0:DH, :], 0.0)
    kps = psB.tile([P, N], F32, tag="sim0", name="kt_m0")
    for k in range(KCH):
        lhsT = cwqk_t[k][:, 0:P]
        for half in range(2):
            nc.tensor.matmul(kps[:, half * COLS:(half + 1) * COLS], lhsT,
                             ct[k][:, half * COLS:(half + 1) * COLS],
                             start=(k == 0), stop=(k == KCH - 1))
    nc.vector.tensor_copy(kpa[0:DH, :], kps[0:DH, :])
    nc.vector.tensor_copy(kpb[DH:P, :], kps[DH:P, :])
    for m in (1, 2, 3):
        res[f"kt{m}"] = projT_m_items(ct, cwqk_t, KT, m, "kt")
        drip.append(f"kt{m}")

    # ---------------- phase B ----------------
    u_sb = [None] * KCH
    w_sb = [None] * KCH

    def norm(p, orient, hh, acc):
        """acc: [65,N] psum; rows 0:64 data, row 64 normalizer."""
        slot = 4 * p + 2 * orient + hh
        rst = pn1.tile([DH + 1, N], F32, tag="rst")
        nc.vector.tensor_copy(rst[:], acc[0:DH + 1, :])
        nc.sync.dma_start(T["normd"][slot:slot + 1, :], rst[DH:DH + 1, :])
        rbc = pn.tile([DH, N], F32, tag="rbc")
        nc.sync.dma_start(
            rbc[:], T["normd"][slot:slot + 1, :].to_broadcast((DH, N)))
        nc.vector.reciprocal_approx_fast(rbc[:], rbc[:])
        ubf = pn.tile([DH, N], BF16, tag="ubf")
        nc.vector.tensor_mul(ubf[:], rst[0:DH, :], rbc[:])
        r0 = orient * 128 + hh * DH
        nc.sync.dma_start(T["uwl"][p][r0:r0 + DH, :], ubf[:])

    def block(p, orient, hh, pending, nfill=2):
        """One (pair, orientation, head) unit: 8 sim+exp steps with the
        U/W ladder at lag 2 and fillers padding the PE.  The tail
        (last two ladder steps + norm) is returned as a closure and runs
        inside the NEXT block's first step, so the exp(7) -> ladder(7)
        chain never delays the next block's first sim/exp."""
        h = 2 * p + hh
        require(f"kt{p}")
        if orient == 0:   # U: simT (j on partitions), ladder vs CV
            lhsT_full, rhs_full = KT[p][hh], QT[p][hh]
            lad = CV
        else:             # W: sim (i on partitions), ladder vs V
            lhsT_full, rhs_full = QT[p][hh], KT[p][hh]
            lad = V
        acc = psB.tile([P, N], F32, tag="uw", name=f"uw{p}_{orient}_{hh}")
        E = [None] * ICH

        def ladder(jc):
            require(f"{'cv' if orient == 0 else 'v'}{p}_{jc}")
            lt = lad[jc][:, h * hs:(h + 1) * hs]
            for half in range(2):
                nc.tensor.matmul(
                    acc[0:hs, half * COLS:(half + 1) * COLS],
                    lt, E[jc][:, half * COLS:(half + 1) * COLS],
                    start=(jc == 0), stop=(jc == ICH - 1))

        for jc in range(ICH):
            ps = psB.tile([P, N], F32, tag=f"sim{jc & 1}")
            lhsT = lhsT_full[:, jc * P:(jc + 1) * P]
            for half in range(2):
                nc.tensor.matmul(ps[:, half * COLS:(half + 1) * COLS],
                                 lhsT, rhs_full[:, half * COLS:(half + 1) * COLS],
                                 start=True, stop=True)
            e = pe.tile([P, N], BF16, tag="E")
            nc.scalar.activation(e[:], ps[:], EXP, scale=SCALE)
            E[jc] = e
            if jc == 0 and pending is not None:
                pending()
            if jc >= 2:
                ladder(jc - 2)
            emit_fillers(nfill)

        def tail():
            ladder(ICH - 2)
            ladder(ICH - 1)
            norm(p, orient, hh, acc)
        return tail

    def load_pair(p):
        rows = ((0, u_sb, "u", 2 * p), (256, u_sb, "u", 2 * p + 1),
                (128, w_sb, "w", 2 * p), (384, w_sb, "w", 2 * p + 1))
        for row, arr, nm, k in rows:
            t = pu.tile([P, N], BF16, tag=f"{nm}sb{k}", name=f"{nm}sb{k}")
            nc.gpsimd.dma_start(t[:], T["uwa"][p][row:row + P, :])
            arr[k] = t

    # partial output projections (k chunks 0..5, from pairs 0-2) run as
    # pair-3 fillers in the pt/pv psum slots freed by the projections.
    cp_out = [None] * ICH
    cp_ctx = [None] * ICH
    pc = [None]

    def partial_items(ic):
        pso = psB.tile([P, COLS], F32, tag="pt", name=f"pso{ic}")
        cpo = pc[0].tile([P, COLS], F32, tag=f"ocp{ic}", name=f"ocp{ic}")
        cp_out[ic] = cpo

        def mko(k):
            def it():
                nc.tensor.matmul(pso[:], u_sb[k][:, ic * P:(ic + 1) * P],
                                 wout_sb[k][:], start=(k == 0), stop=(k == 5))
                if k == 5:
                    nc.vector.tensor_add(cpo[:], pso[:], bout_bc[:])
            return it
        return [mko(k) for k in range(6)]

    def chain(tail, post):
        def f():
            tail()
            post()
        return f

    def post_u3():
        nc.gpsimd.collective_compute(
            "AllGather", mybir.AluOpType.bypass,
            replica_groups=GROUPS,
            ins=[T["uwl"][3][0:128, :]],
            outs=[T["uwa3u"][:]],
        )
        for j in range(2):
            t = pu.tile([P, N], BF16, tag=f"usb{6 + j}", name=f"usb{6 + j}")
            nc.gpsimd.dma_start(t[:], T["uwa3u"][j * P:(j + 1) * P, :])
            u_sb[6 + j] = t

    def post_w3():
        nc.gpsimd.collective_compute(
            "AllGather", mybir.AluOpType.bypass,
            replica_groups=GROUPS,
            ins=[T["uwl"][3][128:256, :]],
            outs=[T["uwa3w"][:]],
        )
        for j in range(2):
            t = pu.tile([P, N], BF16, tag=f"wsb{6 + j}", name=f"wsb{6 + j}")
            nc.gpsimd.dma_start(t[:], T["uwa3w"][j * P:(j + 1) * P, :])
            w_sb[6 + j] = t

    def make_post_pair(p):
        def f():
            nc.gpsimd.collective_compute(
                "AllGather", mybir.AluOpType.bypass,
                replica_groups=GROUPS,
                ins=[T["uwl"][p][:]],
                outs=[T["uwa"][p][:]],
            )
            load_pair(p)
            if p == 2:
                # projections are done by now; free their input pools and
                # queue the partial output projections as pair-3 fillers.
                drain_fillers()
                iostack.close()
                pc[0] = stack.enter_context(tc.tile_pool(name="pc", bufs=1))
                for ic in range(ICH):
                    res[f"op{ic}"] = partial_items(ic)
                    drip.append(f"op{ic}")
        return f

    pending = None
    for p in range(PAIRS):
        nf = 2 if p == 0 else (3 if p < 3 else 4)
        pending = block(p, 0, 0, pending, nf)
        pending = block(p, 0, 1, pending, nf)
        if p == 3:
            pending = chain(pending, post_u3)
        pending = block(p, 1, 0, pending, nf)
        pending = block(p, 1, 1, pending, nf)
        pending = chain(pending, make_post_pair(p) if p < 3 else post_w3)
    pending()

    drain_fillers()
    bstack.close()   # free psB PSUM for the finals

    # ---------------- finals: ctx partials shadow the last gather ----------
    with tc.tile_pool(name="psD", bufs=4, space="PSUM") as psD:
        for ic in range(ICH):
            psc = psD.tile([P, COLS], F32, tag="od", name=f"cpp{ic}")
            for k in range(6):
                nc.tensor.matmul(psc[:], w_sb[k][:, ic * P:(ic + 1) * P],
                                 cwout_sb[k][:], start=(k == 0), stop=(k == 5))
            cpc = pc[0].tile([P, COLS], F32, tag=f"ccp{ic}", name=f"ccp{ic}")
            nc.vector.tensor_add(cpc[:], psc[:], cbout_bc[:])
            cp_ctx[ic] = cpc
            ps = psD.tile([P, COLS], F32, tag="od", name=f"outf{ic}")
            for k in (6, 7):
                nc.tensor.matmul(ps[:], u_sb[k][:, ic * P:(ic + 1) * P],
                                 wout_sb[k][:], start=(k == 6), stop=(k == 7))
            o = pc[0].tile([P, COLS], F32, tag=f"ot{ic % 4}", name=f"oo{ic}")
            nc.vector.tensor_add(o[:], ps[:], cp_out[ic][:])
            (nc.sync if ic % 2 else nc.scalar).dma_start(
                T["out_cols"][ic * P:(ic + 1) * P, :], o[:])
        for ic in range(ICH):
            ps = psD.tile([P, COLS], F32, tag="od", name=f"ctxf{ic}")
            for k in (6, 7):
                nc.tensor.matmul(ps[:], w_sb[k][:, ic * P:(ic + 1) * P],
                                 cwout_sb[k][:], start=(k == 6), stop=(k == 7))
            o = pc[0].tile([P, COLS], F32, tag=f"ot{ic % 4}", name=f"co{ic}")
            nc.vector.tensor_add(o[:], ps[:], cp_ctx[ic][:])
            (nc.sync if ic % 2 else nc.scalar).dma_start(
                T["ctx_cols"][ic * P:(ic + 1) * P, :], o[:])
    stack.close()


def _get_nc():
    global _CACHED_NC
    if _CACHED_NC is None:
        _CACHED_NC = _build_nc()
    return _CACHED_NC


def _reorder_rows(w):
    """Reorder [INNER, :] rows to the uw_all K-chunk order (p-major, group X)."""
    chunks = []
    for p in range(4):
        for X in range(2):
            chunks.append(w[X * 512 + p * 128:X * 512 + (p + 1) * 128])
    return np.concatenate(chunks, axis=0)


def kernel(x, context, w_qk, w_v, cw_qk, cw_v, w_out, b_out, cw_out, cb_out):
    x = np.asarray(x, dtype=np.float32)
    context = np.asarray(context, dtype=np.float32)
    w_qk = np.asarray(w_qk, dtype=np.float32)
    w_v = np.asarray(w_v, dtype=np.float32)
    cw_qk = np.asarray(cw_qk, dtype=np.float32)
    cw_v = np.asarray(cw_v, dtype=np.float32)
    w_out_r = _reorder_rows(np.asarray(w_out, dtype=np.float32)).astype(ml_dtypes.bfloat16)
    cw_out_r = _reorder_rows(np.asarray(cw_out, dtype=np.float32)).astype(ml_dtypes.bfloat16)
    b_out = np.asarray(b_out, dtype=np.float32)
    cb_out = np.asarray(cb_out, dtype=np.float32)

    in_maps = []
    for c in range(8):
        b, g = c // 2, c % 2
        sl = slice(g * IL, (g + 1) * IL)
        in_maps.append({
            "xT": np.ascontiguousarray(x[b].T).astype(ml_dtypes.bfloat16),
            "ctxT": np.ascontiguousarray(context[b].T).astype(ml_dtypes.bfloat16),
            "wqk": np.ascontiguousarray(w_qk[:, sl]).astype(ml_dtypes.bfloat16),
            "wv": np.ascontiguousarray(w_v[:, sl]).astype(ml_dtypes.bfloat16),
            "cwqk": np.ascontiguousarray(cw_qk[:, sl]).astype(ml_dtypes.bfloat16),
            "cwv": np.ascontiguousarray(cw_v[:, sl]).astype(ml_dtypes.bfloat16),
            "wout": np.ascontiguousarray(w_out_r[:, sl]),
            "cwout": np.ascontiguousarray(cw_out_r[:, sl]),
            "bout": np.ascontiguousarray(b_out[None, sl]),
            "cbout": np.ascontiguousarray(cb_out[None, sl]),
        })

    nc = _get_nc()
    res = run_bass_kernel_spmd(nc, in_maps, list(range(8)))

    out = np.empty((B, N, DIM), dtype=np.float32)
    ctx_out = np.empty((B, N, DIM), dtype=np.float32)
    for b in range(B):
        out[b, :, 0:COLS] = res.results[2 * b]["out_cols"]
        out[b, :, COLS:] = res.results[2 * b + 1]["out_cols"]
        ctx_out[b, :, 0:COLS] = res.results[2 * b]["ctx_cols"]
        ctx_out[b, :, COLS:] = res.results[2 * b + 1]["ctx_cols"]
    return out, ctx_out
